# revision 1
# baseline (speedup 1.0000x reference)
"""Paged-KV varlen causal GQA attention for Trainium2, sharded over 8 NeuronCores.

Problem (hardcoded from spec): T=4096 tokens, 16 q heads / 8 kv heads, head_dim=64,
scale=0.125. k/v are scattered into paged caches via slot_mapping, read back, and
causal varlen attention (segments from cu_seqlens) is computed.

Sharding: tensor-parallel over kv heads -- core h gets kv head h and its 2 GQA
query heads. slot_mapping / cu_seqlens handled on host (index math only).

Device kernel (per core), per causal segment:
  sT[keys, queries] = kT.T @ qT        (PE; both q heads concurrently via row tiling, K=64)
  w = exp(0.125 * sT)                  (ScalarE, PSUM->SBUF bf16; no max-subtraction --
                                        scores are O(5), exp stays in fp32 range)
  diagonal 128-tiles: w *= upper-tri mask (VectorE)
  oT[65, q] += [v | 1].T @ w           (PE accumulate; row 64 = softmax denominator)
  o[q, 0:64] = oT.T / oT.T[:, 64]      (PE transpose + VectorE reciprocal/scale)
"""

import os
from contextlib import ExitStack
from math import ceil

import numpy as np
import ml_dtypes

import concourse.bass as bass
import concourse.mybir as mybir
import concourse.tile as tile
from concourse import bacc
from concourse.bass_utils import run_bass_kernel_spmd
from concourse.masks import make_identity

NKV = 8
G = 2
D = 64
SCALE = 0.125

# test.py pokes these for profiling
TRACE = bool(int(os.environ.get("KERNEL_TRACE", "0")))
LAST_RESULT = None

_PROGRAM_CACHE = {}


def _build_program(T, segments):
    f32 = mybir.dt.float32
    bf16 = mybir.dt.bfloat16

    nc = bacc.Bacc(
        "TRN2",
        target_bir_lowering=False,
        debug=False,
        enable_asserts=False,
        num_devices=8,
    )
    qT_d = nc.dram_tensor("qT", [128, T], bf16, kind="ExternalInput").ap()
    kT_d = nc.dram_tensor("kT", [64, T], bf16, kind="ExternalInput").ap()
    v_d = nc.dram_tensor("v", [T, D], bf16, kind="ExternalInput").ap()
    o_d = nc.dram_tensor("o", [T, 2 * D], f32, kind="ExternalOutput").ap()

    with tile.TileContext(nc) as tc, ExitStack() as ctx:
        const = ctx.enter_context(tc.tile_pool(name="const", bufs=1))
        qkpool = ctx.enter_context(tc.tile_pool(name="qk", bufs=1))
        vpool = ctx.enter_context(tc.tile_pool(name="vt", bufs=4))
        spool = ctx.enter_context(tc.tile_pool(name="se", bufs=6))
        opool = ctx.enter_context(tc.tile_pool(name="osb", bufs=4))
        ps_o = ctx.enter_context(tc.tile_pool(name="ps_o", bufs=1, space="PSUM"))
        ps_s = ctx.enter_context(tc.tile_pool(name="ps_s", bufs=3, space="PSUM"))

        ident = const.tile([128, 128], f32)
        make_identity(nc, ident)
        # trimask[p, c] = 1 if c >= p else 0 (valid = query col >= key partition)
        trimask = const.tile([128, 128], bf16)
        nc.gpsimd.memset(trimask, 0.0)
        nc.gpsimd.affine_select(
            out=trimask,
            in_=trimask,
            compare_op=mybir.AluOpType.is_gt,
            fill=1.0,
            base=0,
            pattern=[[-1, 128]],
            channel_multiplier=1,
        )

        qT = qkpool.tile([128, T], bf16)
        # kT duplicated on partitions 0-63 and 64-127 so the two q heads'
        # QK matmuls can run in different PE row-groups concurrently.
        kT = qkpool.tile([128, T], bf16)

        def finalize_range(s0, q0, h, m, b0, c_lo, c_hi, t65, tp_tag, act_copy, uniq):
            """Transpose + normalize t65 cols [c_lo, c_hi) (128-aligned c_lo) and
            DMA the rows out. t65 already holds the PSUM copy for that range."""
            nchunk = ceil((c_hi - c_lo) / 128)
            tp = ps_o.tile(
                [128, 65 * nchunk], f32, tag=tp_tag, name=f"tp_{uniq}"
            )
            for j in range(nchunk):
                n = min(128, c_hi - c_lo - j * 128)
                nc.tensor.transpose(
                    tp[:n, 65 * j : 65 * j + 65],
                    t65[:65, c_lo + j * 128 : c_lo + j * 128 + n],
                    ident[:65, :65],
                )
            rcp = opool.tile([128, nchunk], f32, tag="rcp", name=f"rcp_{uniq}")
            osb = opool.tile([128, D * nchunk], f32, tag="osb", name=f"osb_{uniq}")
            nf = (c_hi - c_lo) // 128  # chunks with all 128 rows written
            if nf:
                nc.vector.reciprocal(rcp[:, :nf], tp[:, D : 65 * nf : 65])
                tp_v = tp[:, : 65 * nf].rearrange("p (c k) -> p c k", k=65)[:, :, 0:D]
                rcp_v, tp_v = bass.broadcast_tensor_aps(
                    rcp[:, :nf].rearrange("p (c k) -> p c k", k=1), tp_v
                )
                nc.vector.tensor_mul(
                    osb[:, : D * nf].rearrange("p (c k) -> p c k", k=D), tp_v, rcp_v
                )
            if nf < nchunk:
                # partial last chunk: only n rows of tp were written -- use
                # exact-row ops so nothing stale is read
                n = (c_hi - c_lo) - nf * 128
                nc.vector.reciprocal(
                    rcp[:n, nf : nf + 1], tp[:n, 65 * nf + D : 65 * nf + D + 1]
                )
                nc.vector.tensor_scalar_mul(
                    osb[:n, D * nf : D * nf + D],
                    tp[:n, 65 * nf : 65 * nf + D],
                    rcp[:n, nf : nf + 1],
                )
            # tail-range output DMAs (act_copy) go out on the Activation
            # HWDGE queue, which is idle once the exp stream has drained --
            # their generation spans overlap the SP queue's instead of
            # serializing behind them
            r0 = s0 + q0 + b0 + c_lo
            span = c_hi - c_lo
            cfull = span // 128
            dmae = nc.sync
            if cfull:
                dmae.dma_start(
                    o_d[r0 : r0 + cfull * 128, D * h : D * h + D].rearrange(
                        "(c p) k -> p c k", p=128
                    ),
                    osb.rearrange("p (c k) -> p c k", k=D)[:, :cfull, :],
                )
            if span % 128:
                n = span % 128
                dmae.dma_start(
                    o_d[r0 + cfull * 128 : r0 + span, D * h : D * h + D],
                    osb[:n, D * cfull : D * cfull + D],
                )

        def finalize_block(s0, q0, h, m, b0, blen, oT, act_copy=False):
            """Transpose + normalize one [65, blen] oT block and DMA it out."""
            uniq = f"{s0}_{q0}_{h}_{m}"
            t65 = opool.tile([65, 512], f32, tag="t65", name=f"t65_{uniq}")
            if act_copy:
                nc.scalar.copy(t65[:, :blen], oT)
            else:
                nc.vector.tensor_copy(t65[:, :blen], oT)
            finalize_range(
                s0, q0, h, m, b0, 0, blen, t65, f"oT_{m}", act_copy, uniq
            )

        supers = []
        for (s0, s1) in segments:
            L = s1 - s0
            for q0 in range(0, L, 1024):
                supers.append((s0, s1, q0, min(1024, L - q0)))

        seen_seg = set()
        pending_final = None
        for si, (s0, s1, q0, qlen) in enumerate(supers):
            is_last_super = si == len(supers) - 1
            if s0 not in seen_seg:
                seen_seg.add(s0)
                if si == 0:
                    # first segment: order + split DMAs so head-0's first QK
                    # unblocks as early as possible
                    nc.sync.dma_start(kT[0:64, s0:s1], kT_d[:, s0:s1])
                    nc.sync.dma_start(qT[0:64, s0:s1], qT_d[0:64, s0:s1])
                    nc.sync.dma_start(kT[64:128, s0:s1], kT_d[:, s0:s1])
                    nc.sync.dma_start(qT[64:128, s0:s1], qT_d[64:128, s0:s1])
                else:
                    nc.sync.dma_start(qT[:, s0:s1], qT_d[:, s0:s1])
                    nc.sync.dma_start(kT[0:64, s0:s1], kT_d[:, s0:s1])
                    nc.sync.dma_start(kT[64:128, s0:s1], kT_d[:, s0:s1])
            edges = list(range(0, qlen, 512)) + [qlen]
            nblk = len(edges) - 1
            kend = q0 + qlen
            nkt = ceil(kend / 128)
            # stage all v tiles for this super-block in one DMA; col 64 of
            # each [128, 65] tile is the ones column for the denominator row
            vst = vpool.tile([128, nkt, D + 1], bf16, tag="vt")
            nfull = kend // 128
            if nfull:
                nc.sync.dma_start(
                    vst[:, :nfull, 0:D],
                    v_d[s0 : s0 + nfull * 128, :].rearrange("(n p) d -> p n d", p=128),
                )
            if kend % 128:
                rem = kend % 128
                nc.sync.dma_start(
                    vst[:rem, nfull, 0:D], v_d[s0 + nfull * 128 : s0 + kend, :]
                )
            nc.any.memset(vst[:, :, D : D + 1], 1.0)
            # heads processed sequentially: halves the live oT accumulators
            # (2 PSUM banks instead of 4) so the score pool gets 3 slots --
            # letting PE run further ahead of the ScalarE exp bottleneck
            for h in range(G):
                oT_ps = [
                    ps_o.tile(
                        [65, edges[m + 1] - edges[m]],
                        f32,
                        tag=f"oT_{m}",
                        name=f"oT_{s0}_{q0}_{h}_{m}",
                    )
                    for m in range(nblk)
                ]
                def kt_info(kt):
                    klo = kt * 128
                    kp = min(128, kend - klo)
                    c0 = max(q0, klo)
                    return klo, kp, c0, kend - c0

                # pack kts into score-tile groups: big kts (span beyond the
                # last 512-block) stay singletons in the 2-bank "spb" rotation;
                # consecutive full small kts merge into one tile (and one exp)
                # as long as each member's span stays inside a PSUM bank
                groups = []  # list[list[(kt, off)]] -- small groups first-fit
                small_groups = []
                for kt in range(nkt):
                    klo, kp, c0, span = kt_info(kt)
                    small = (c0 - q0) >= edges[nblk - 1]
                    if not small:
                        groups.append([(kt, 0)])
                        continue
                    placed = False
                    # the final pass keeps strict kt order (consecutive merges
                    # only) so the two-stage tail finalize stays valid
                    allow_ff = not (is_last_super and h == G - 1)
                    if kp == 128:
                        for g in small_groups:
                            if kt_info(g[0][0])[1] != 128:
                                continue
                            if not allow_ff and (
                                g is not small_groups[-1] or g[-1][0] != kt - 1
                            ):
                                continue
                            off = g[-1][1] + kt_info(g[-1][0])[3]
                            if (off % 512) + span <= 512 and off + span <= 512:
                                g.append((kt, off))
                                placed = True
                                break
                    if not placed:
                        g = [(kt, 0)]
                        small_groups.append(g)
                        groups.append(g)

                # last EMITTED kt that writes each block (emission order can
                # differ from kt order after first-fit packing) -- keys the
                # accumulation-group stop flag and finalize triggers
                last_emit = {}
                for g in groups:
                    for kt, _ in g:
                        klo = kt * 128
                        c0 = max(q0, klo)
                        for m in range(nblk):
                            if max(q0 + edges[m], c0) < q0 + edges[m + 1]:
                                last_emit[m] = kt
                n_small = 0
                for gi, members in enumerate(groups):
                    g_kt0 = members[0][0]
                    g_klo0, g_kp0, g_c00, g_span0 = kt_info(g_kt0)
                    g_small = (g_c00 - q0) >= edges[nblk - 1]
                    if g_small:
                        lk, loff = members[-1]
                        total = loff + kt_info(lk)[3]
                        sp = ps_s.tile([128, 512], f32, tag="sps", bufs=2)
                        n_small += 1
                        rows = 128 if len(members) > 1 else g_kp0
                    else:
                        sp = ps_s.tile([128, 1024], f32, tag="spb", bufs=2)
                        total = g_span0
                        rows = g_kp0
                    for kt, off in members:
                        klo, kp, c0, span = kt_info(kt)
                        for m in range(nblk):
                            b0 = q0 + edges[m]
                            b1 = q0 + edges[m + 1]
                            a0 = max(b0, c0)
                            if a0 >= b1:
                                continue
                            lo = (off + a0 - c0) if g_small else (a0 - q0)
                            nc.tensor.matmul(
                                sp[:kp, lo : lo + b1 - a0],
                                kT[64 * h : 64 * h + 64, s0 + klo : s0 + klo + kp],
                                qT[64 * h : 64 * h + 64, s0 + a0 : s0 + b1],
                                start=True,
                                stop=True,
                                tile_position=(64 * h, 0),
                            )
                    if g_kt0 == 0 and pending_final is not None:
                        # deferred finalize of the previous pass's last block:
                        # emitted after this pass's first QK so PE feeds the
                        # ScalarE exp pipeline before the finalize transposes
                        finalize_block(*pending_final)
                        pending_final = None
                    se = spool.tile([128, 1024], bf16, tag="se")
                    if si == 0 and h == 0 and g_kt0 == 0 and not g_small:
                        # kernel warmup: exp per block so the first exp only
                        # waits on the first QK matmul
                        for m in range(nblk):
                            e0 = max(q0 + edges[m], g_c00) - g_c00
                            e1 = q0 + edges[m + 1] - g_c00
                            if e0 >= e1:
                                continue
                            nc.scalar.activation(
                                se[:rows, e0:e1],
                                sp[:rows, g_c00 - q0 + e0 : g_c00 - q0 + e1],
                                mybir.ActivationFunctionType.Exp,
                                scale=SCALE,
                            )
                            if m == 0 and g_c00 == g_klo0:
                                dn = min(g_kp0, g_span0)
                                nc.vector.tensor_mul(
                                    se[:rows, :dn], se[:rows, :dn], trimask[:rows, :dn]
                                )
                    else:
                        elo = 0 if g_small else g_c00 - q0
                        nc.scalar.activation(
                            se[:rows, :total],
                            sp[:rows, elo : elo + total],
                            mybir.ActivationFunctionType.Exp,
                            scale=SCALE,
                        )
                        for kt, off in members:
                            klo, kp, c0, span = kt_info(kt)
                            if c0 == klo:
                                # diagonal tile: causal mask on this member
                                dn = min(kp, span)
                                so = off if g_small else 0
                                nc.vector.tensor_mul(
                                    se[:kp, so : so + dn],
                                    se[:kp, so : so + dn],
                                    trimask[:kp, :dn],
                                )
                    for kt, off in members:
                        klo, kp, c0, span = kt_info(kt)
                        for m in range(nblk):
                            b0 = q0 + edges[m]
                            b1 = q0 + edges[m + 1]
                            a0 = max(b0, c0)
                            if a0 >= b1:
                                continue
                            blen = edges[m + 1] - edges[m]
                            last = kt == last_emit[m]
                            tail2 = (
                                is_last_super
                                and h == G - 1
                                and m == nblk - 1
                                and nkt >= 2
                            )
                            c_split = 128 * (nkt - 1) - (q0 + edges[m])
                            split2 = tail2 and 128 <= c_split < blen
                            stage1_here = split2 and kt == nkt - 2
                            so = off if g_small else (c0 - g_c00 if False else 0)
                            ro = (off + a0 - c0) if g_small else (a0 - g_c00)
                            nc.tensor.matmul(
                                oT_ps[m][:, a0 - b0 : b1 - b0],
                                vst[:, kt, :][:kp, : D + 1],
                                se[:kp, ro : ro + b1 - a0],
                                start=(kt == 0),
                                stop=last or stage1_here,
                                skip_group_check=(split2 and kt == nkt - 1),
                            )
                            if stage1_here:
                                uniq = f"{s0}_{q0}_{h}_{m}_s1"
                                t65s = opool.tile(
                                    [65, 512], f32, tag="t65", name=f"t65_{uniq}"
                                )
                                nc.vector.tensor_copy(
                                    t65s[:, :c_split], oT_ps[m][:, :c_split]
                                )
                                other = m - 1 if m >= 1 else m + 1
                                finalize_range(
                                    s0, q0, h, m, edges[m], 0, c_split,
                                    t65s, f"oT_{other}", False, uniq,
                                )
                                tail_t65 = t65s
                            if last:
                                if split2:
                                    uniq = f"{s0}_{q0}_{h}_{m}_s2"
                                    nc.scalar.copy(
                                        tail_t65[:, c_split:blen],
                                        oT_ps[m][:, c_split:blen],
                                    )
                                    finalize_range(
                                        s0, q0, h, m, edges[m], c_split, blen,
                                        tail_t65, f"oT_{m}", True, uniq,
                                    )
                                    continue
                                args = (s0, q0, h, m, edges[m], blen, oT_ps[m])
                                if m == nblk - 1:
                                    # closes at pass end: defer past the next
                                    # pass's first QK
                                    pending_final = args
                                else:
                                    finalize_block(*args)

        if pending_final is not None:
            finalize_block(*pending_final, act_copy=True)
            pending_final = None

    nc.compile()
    return nc


def _segments_from_cu(cu_seqlens, T):
    edges = sorted(set([0, T] + [int(c) for c in cu_seqlens if 0 < int(c) < T]))
    return [(edges[i], edges[i + 1]) for i in range(len(edges) - 1)]


def kernel(q, k, v, k_cache, v_cache, slot_mapping, cu_seqlens):
    global LAST_RESULT
    T = q.shape[0]
    nslots = k_cache.shape[0]

    # Emulate scatter-then-gather through the paged cache: for duplicate slots
    # the last writer wins, so token i reads back k[lastw[slot[i]]].
    slot = np.asarray(slot_mapping, dtype=np.int64)
    lastw = np.zeros(nslots, dtype=np.int64)
    lastw[slot] = np.arange(T)
    lw = lastw[slot]
    k_eff = np.asarray(k)[lw]
    v_eff = np.asarray(v)[lw]

    segments = _segments_from_cu(np.asarray(cu_seqlens), T)
    key = (T, tuple(segments))
    if key not in _PROGRAM_CACHE:
        _PROGRAM_CACHE[key] = _build_program(T, segments)
    nc = _PROGRAM_CACHE[key]

    bf = ml_dtypes.bfloat16
    qh = np.ascontiguousarray(
        np.asarray(q).reshape(T, NKV * G, D).transpose(1, 2, 0)
    ).astype(bf)  # [16, 64, T]
    kh = np.ascontiguousarray(k_eff.reshape(T, NKV, D).transpose(1, 2, 0)).astype(bf)
    vh = v_eff.reshape(T, NKV, D).astype(bf)  # [T, 8, 64]

    in_maps = [
        {
            "qT": np.ascontiguousarray(qh[2 * h : 2 * h + 2].reshape(128, T)),
            "kT": np.ascontiguousarray(kh[h]),
            "v": np.ascontiguousarray(vh[:, h, :]),
        }
        for h in range(NKV)
    ]

    res = run_bass_kernel_spmd(nc, in_maps, core_ids=list(range(8)), trace=TRACE)
    LAST_RESULT = res

    out = np.empty((T, NKV * G * D), dtype=np.float32)
    ov = out.reshape(T, NKV, G * D)
    for h in range(NKV):
        ov[:, h, :] = res.results[h]["o"]
    return out



# revision 18
# speedup vs baseline: 1.2401x; 1.2401x over previous
"""Paged-KV varlen causal GQA attention for Trainium2, sharded over 8 NeuronCores.

Problem (hardcoded from spec): T=4096 tokens, 16 q heads / 8 kv heads, head_dim=64,
scale=0.125. k/v are scattered into paged caches via slot_mapping, read back, and
causal varlen attention (segments from cu_seqlens) is computed.

Sharding: tensor-parallel over kv heads -- core h gets kv head h and its 2 GQA
query heads. slot_mapping / cu_seqlens handled on host (index math only).

Device kernel (per core): a single flat stream of score tiles [128, <=1536] f32
(2 PSUM buffers x 3 banks) packed with (head, key-tile) pieces:
  sp[keys, queries] = kT.T @ qT          (PE, per piece, bank-split)
  se = exp(0.125 * sp)                   (ScalarE, one exp per score tile -- the
                                          bottleneck engine; tiles are as wide as
                                          PSUM allows to amortize instr overhead)
  diagonal blocks: se *= trimask         (VectorE, one batched mul per block)
  o[q, 0:65] += se[:,qc].T @ [v | 1]     (PE with se as STATIONARY: output lands
                                          query-major, col 64 = softmax denom --
                                          no transposes / PSUM->SBUF copies)
  osb[q, d] = o[q, d] * 1/o[q, 64]       (VectorE reciprocal + broadcast mul)
"""

import os
from contextlib import ExitStack
from math import ceil

import numpy as np
import ml_dtypes

import concourse.bass as bass
import concourse.mybir as mybir
import concourse.tile as tile
from concourse import bacc
from concourse.bass_utils import run_bass_kernel_spmd

NKV = 8
G = 2
D = 64
SCALE = 0.125

TILE_W = 1536  # score tile width (3 PSUM banks of f32)
BANK = 512     # f32 cols per PSUM bank
W0Q = 1024     # warmup tensor covers kT[0:128] + qT[128:1024] of segment 0

# test.py pokes these for profiling
TRACE = bool(int(os.environ.get("KERNEL_TRACE", "0")))
LAST_RESULT = None

_PROGRAM_CACHE = {}


def _plan(segments):
    """Pack the per-(segment, head) score streams into a flat list of score
    tiles. Returns (tiles, n_keytiles_total).

    Each tile dict:
      sz: used cols
      qk: [(h, s0, klo, kp, qa, qb, col, from_w0)]   QK subpieces (bank-split)
      masks: [(col, nfull)]                          batched trimask ranges
      pv: [(h, si, qc, col, cw, ktg, kp, start, stop)]
      norms: [(h, si, f, qc0, nq)]                   norm groups due after this tile
      dmas: [(si, c0, c1)]                           output row-blocks due
    """
    tiles = []
    cur = None
    chains = {}  # (h, si, qc) -> [ [tile_idx, col, cw, ktg, kp], ... ]
    cap_now = TILE_W

    def new_tile():
        nonlocal cur
        cur = {"sz": 0, "qk": [], "masks": [], "pv": [], "norms": [], "dmas": []}
        tiles.append(cur)

    new_tile()

    # global key-tile counter (vt layout is per-segment tiled on host)
    seg_ktg0 = []
    nkt_tot = 0
    for (s0, s1) in segments:
        seg_ktg0.append(nkt_tot)
        nkt_tot += ceil((s1 - s0) / 128)

    def place_span(h, si, kt, qa, qb, w0_ok):
        """Place one query span of key tile kt, splitting at tile caps and
        PSUM bank edges; register PV chunks."""
        nonlocal cur
        s0, s1 = segments[si]
        L = s1 - s0
        klo = 128 * kt
        kp = min(128, L - klo)
        ktg = seg_ktg0[si] + kt
        q = qa
        while q < qb:
            if cur["sz"] >= cap_now:
                new_tile()
            room = cap_now - cur["sz"]
            take = min(qb - q, room)
            col = cur["sz"]
            # QK subpieces: split at bank edges and at the w0 boundary
            qq = q
            while qq < q + take:
                c = col + (qq - q)
                lim = qq + min(q + take - qq, BANK - (c % BANK))
                use_w0 = w0_ok and 128 <= qq < W0Q
                if use_w0 and lim > W0Q:
                    lim = W0Q
                cur["qk"].append((h, s0, klo, kp, qq, lim, c, use_w0))
                qq = lim
            # PV chunks (128-query aligned within the segment)
            for qs in range(q, q + take, 128):
                qc = qs // 128
                cw = min(128, qb - qs, q + take - qs)
                chains.setdefault((h, si, qc), []).append(
                    [len(tiles) - 1, col + (qs - q), cw, ktg, kp]
                )
            cur["sz"] += take
            if cur["sz"] % 128:  # ragged tail: re-align for bank safety
                cur["sz"] += 128 - cur["sz"] % 128
            q += take

    def place_diag_run(h, si, kts, w0_ok):
        """Place diagonal chunks contiguously with one batched mask entry."""
        nonlocal cur
        s0, s1 = segments[si]
        L = s1 - s0
        total = sum(min(128, L - 128 * kt) for kt in kts)
        if cur["sz"] + total > cap_now and cur["sz"] > 0:
            new_tile()
        col0 = cur["sz"]
        for kt in kts:
            klo = 128 * kt
            kp = min(128, L - klo)
            place_span(h, si, kt, klo, klo + kp, w0_ok)
        cur["masks"].append((col0, (cur["sz"] - col0) // 128))

    nseg = len(segments)
    for si, (s0, s1) in enumerate(segments):
        L = s1 - s0
        nkt = ceil(L / 128)
        nqc = ceil(L / 128)
        for h in range(G):
            first_sh = si == 0 and h == 0 and L >= 1024
            last_sh = (
                si == nseg - 1 and h == G - 1 and nqc >= 8 and L % 128 == 0
            )
            nd = [kt for kt in range(nkt - 1) if 128 * (kt + 1) < L]
            if first_sh:
                # warmup: small leading non-diag tiles, diag last
                cap_now = 384
                place_span(h, si, 0, 128, min(512, L), True)
                new_tile()
                cap_now = 512
                place_span(h, si, 0, 512, min(1024, L), True)
                new_tile()
                cap_now = TILE_W
                if L > 1024:
                    place_span(h, si, 0, 1024, L, True)
                for kt in nd[1:]:
                    place_span(h, si, kt, 128 * (kt + 1), L, True)
                place_diag_run(h, si, range(nkt), True)
            elif last_sh:
                # tail: isolate the last 128 queries' pieces in small final
                # tiles so the closing exp->PV->norm->DMA chain is short
                place_diag_run(h, si, range(nkt - 1), False)
                for kt in nd:
                    place_span(h, si, kt, 128 * (kt + 1), L - 128, False)
                if cur["sz"] > 0:
                    new_tile()
                for kt in nd[:-2]:
                    place_span(h, si, kt, L - 128, L, False)
                if cur["sz"] > 0:
                    new_tile()
                for kt in nd[-2:]:
                    place_span(h, si, kt, L - 128, L, False)
                place_diag_run(h, si, [nkt - 1], False)
            else:
                place_diag_run(h, si, range(nkt), False)
                for kt in nd:
                    place_span(h, si, kt, 128 * (kt + 1), L, False)

    # qchunk -> oh-bank mapping: 4 chunks per PSUM bank, except the last
    # segment where the final chunk gets its own bank so its accumulation
    # group closes (and is normalized/DMAd) independently of chunks 4..nqc-2
    def oh_banks(si):
        L = segments[si][1] - segments[si][0]
        nqc = ceil(L / 128)
        if si == nseg - 1 and nqc >= 6:
            return [list(range(0, 4)), list(range(4, nqc - 1)), [nqc - 1]]
        return [
            list(range(4 * f, min(4 * f + 4, nqc))) for f in range(ceil(nqc / 4))
        ]

    qc_bank = {}  # (si, qc) -> (f, fbase)
    for si in range(nseg):
        for f, qcs in enumerate(oh_banks(si)):
            for qc in qcs:
                qc_bank[(si, qc)] = (f, qcs[0])

    # start/stop flags are PER PSUM BANK (zero region), not per qchunk chain:
    # start_tensor_calc pends-zero the whole 2KB bank, so only the
    # chronologically first matmul into an oh tile may set it, and only the
    # last sets stop. Per-byte lazy zeroing makes each chain's first write a
    # plain store and later writes accumulates, regardless of interleaving.
    oh_groups = {}
    for (h, si, qc), ch in chains.items():
        for e in ch:
            oh_groups.setdefault((h, si, qc_bank[(si, qc)][0]), []).append(e)
    for grp in oh_groups.values():
        grp.sort(key=lambda e: (e[0], e[1]))
        for e in grp:
            e.append(e is grp[0])
            e.append(e is grp[-1])
    for (h, si, qc), ch in chains.items():
        for e in ch:
            tiles[e[0]]["pv"].append((h, si, qc, e[1], e[2], e[3], e[4], e[5], e[6]))

    # order pv entries in each tile by emission col to keep per-chain order
    for t in tiles:
        t["pv"].sort(key=lambda p: (p[3], p[0], p[1], p[2]))

    # norm groups (one per oh bank): due after max last tile over the chains
    for si in range(nseg):
        due_rows = {}
        banks = oh_banks(si)
        for f, gq in enumerate(banks):
            for h in range(G):
                last = max(chains[(h, si, qc)][-1][0] for qc in gq)
                tiles[last]["norms"].append((h, si, f, gq[0], len(gq)))
                for qc in gq:
                    due_rows[(h, qc)] = last
        # output DMAs: per norm group for the last segment, else one per seg
        if len(banks) > 2:
            for gq in banks:
                due = max(due_rows[(h, qc)] for h in range(G) for qc in gq)
                tiles[due]["dmas"].append((si, gq[0], gq[-1] + 1))
        else:
            due = max(due_rows.values())
            nqc = ceil((segments[si][1] - segments[si][0]) / 128)
            tiles[due]["dmas"].append((si, 0, nqc))

    return tiles, nkt_tot, qc_bank


def _build_program(T, segments):
    f32 = mybir.dt.float32
    bf16 = mybir.dt.bfloat16

    tiles, nkt_tot, qc_bank = _plan(segments)
    L0 = segments[0][1] - segments[0][0]
    use_w0 = L0 >= 1024

    nc = bacc.Bacc(
        "TRN2",
        target_bir_lowering=False,
        debug=False,
        enable_asserts=False,
        num_devices=8,
    )
    qT_d = nc.dram_tensor("qT", [128, T], bf16, kind="ExternalInput").ap()
    kT_d = nc.dram_tensor("kT", [64, T], bf16, kind="ExternalInput").ap()
    vt_d = nc.dram_tensor("vt", [128, nkt_tot * 65], bf16, kind="ExternalInput").ap()
    if use_w0:
        w0_d = nc.dram_tensor("w0", [64, W0Q], bf16, kind="ExternalInput").ap()
    o_d = nc.dram_tensor("o", [T, G * D], f32, kind="ExternalOutput").ap()

    with tile.TileContext(nc) as tc, ExitStack() as ctx:
        const = ctx.enter_context(tc.tile_pool(name="const", bufs=1))
        inpool = ctx.enter_context(tc.tile_pool(name="inp", bufs=1))
        sepool = ctx.enter_context(tc.tile_pool(name="se", bufs=4))
        opool = ctx.enter_context(tc.tile_pool(name="osb", bufs=2))
        rpool = ctx.enter_context(tc.tile_pool(name="rcp", bufs=4))
        ps_s = ctx.enter_context(tc.tile_pool(name="ps_s", bufs=2, space="PSUM"))
        ps_o = ctx.enter_context(tc.tile_pool(name="ps_o", bufs=2, space="PSUM"))

        # trimask[p, c] = 1 if c >= p else 0 (valid = query col >= key partition)
        trimask = const.tile([128, 128], bf16)
        nc.gpsimd.memset(trimask, 0.0)
        nc.gpsimd.affine_select(
            out=trimask,
            in_=trimask,
            compare_op=mybir.AluOpType.is_gt,
            fill=1.0,
            base=0,
            pattern=[[-1, 128]],
            channel_multiplier=1,
        )

        qT = inpool.tile([128, T], bf16)
        kT = inpool.tile([128, T], bf16)
        vt = inpool.tile([128, nkt_tot * 65], bf16)
        vtv = vt.rearrange("p (n k) -> p n k", k=65)
        if use_w0:
            w0 = inpool.tile([64, W0Q], bf16, padded_shape=[128, W0Q])

        # ---- input DMAs: all issued up front ----
        # kT halves are loaded separately (h0 rows 0:64 first) so head 0's QK
        # stream unblocks as early as possible; w0 covers the whole first
        # (seg0, h0) non-diagonal stream so tiles 0..3 only depend on 1-2 DMAs
        s00, s01 = segments[0]
        kg = 0
        if use_w0:
            nc.sync.dma_start(w0, w0_d)
        first = True
        for (s0, s1) in segments:
            nkt = ceil((s1 - s0) / 128)
            nc.sync.dma_start(kT[0:64, s0:s1], kT_d[:, s0:s1])
            if first and use_w0:
                nc.sync.dma_start(
                    vt[:, kg * 65 : (kg + nkt) * 65],
                    vt_d[:, kg * 65 : (kg + nkt) * 65],
                )
                nc.sync.dma_start(qT[:, s0:s1], qT_d[:, s0:s1])
            else:
                nc.sync.dma_start(qT[:, s0:s1], qT_d[:, s0:s1])
                nc.sync.dma_start(
                    vt[:, kg * 65 : (kg + nkt) * 65],
                    vt_d[:, kg * 65 : (kg + nkt) * 65],
                )
            nc.sync.dma_start(kT[64:128, s0:s1], kT_d[:, s0:s1])
            kg += nkt
            first = False

        # ---- main stream ----
        se_tiles = {}
        oh_tiles = {}
        osb_tiles = {}

        def get_oh(h, si, f):
            key = (h, si, f)
            if key not in oh_tiles:
                oh_tiles[key] = ps_o.tile(
                    [128, 512], f32, tag="oh", name=f"oh_{h}_{si}_{f}"
                )
            return oh_tiles[key]

        def get_osb(si):
            if si not in osb_tiles:
                nqc = ceil((segments[si][1] - segments[si][0]) / 128)
                osb_tiles[si] = opool.tile(
                    [128, nqc * 128], f32, tag="osb", name=f"osb_{si}"
                )
            return osb_tiles[si]

        def emit_qk(g, sp):
            for (h, s0, klo, kp, qa, qb, col, from_w0) in tiles[g]["qk"]:
                lhsT = (
                    w0[0:64, 0:kp]
                    if (from_w0 and klo == 0)
                    else kT[64 * h : 64 * h + 64, s0 + klo : s0 + klo + kp]
                )
                rhs = (
                    w0[0:64, qa:qb]
                    if from_w0
                    else qT[64 * h : 64 * h + 64, s0 + qa : s0 + qb]
                )
                nc.tensor.matmul(
                    sp[:kp, col : col + qb - qa],
                    lhsT,
                    rhs,
                    start=True,
                    stop=True,
                    tile_position=(64 * h, 0),
                )

        def emit_post(g):
            t = tiles[g]
            se = se_tiles.pop(g)
            for (col, nfull) in t["masks"]:
                if nfull:
                    sev = se[:, col : col + nfull * 128].rearrange(
                        "p (c k) -> p c k", k=128
                    )
                    triv = trimask.rearrange("p (c k) -> p c k", k=128)
                    triv, sev = bass.broadcast_tensor_aps(triv, sev)
                    nc.vector.tensor_mul(
                        se[:, col : col + nfull * 128].rearrange(
                            "p (c k) -> p c k", k=128
                        ),
                        sev,
                        triv,
                    )
            for (h, si, qc, col, cw, ktg, kp, start, stop) in t["pv"]:
                f, fbase = qc_bank[(si, qc)]
                oh = get_oh(h, si, f)
                lq = qc - fbase
                nc.tensor.matmul(
                    oh[:cw, 128 * lq : 128 * lq + 65],
                    se[:kp, col : col + cw],
                    vtv[:kp, ktg, :],
                    start=start,
                    stop=stop,
                )
            for (h, si, f, qc0, nq) in t["norms"]:
                oh = oh_tiles[(h, si, f)]
                osb = get_osb(si)
                lq = 0  # bank-local offset: each norm group is a whole oh tile
                uniq = f"{h}_{si}_{qc0}"
                rcp = rpool.tile([128, 8], f32, tag="rcp", name=f"rcp_{uniq}")
                nc.vector.reciprocal(
                    rcp[:, :nq],
                    oh[:, 128 * lq + 64 : 128 * (lq + nq - 1) + 65 : 128],
                )
                ohv = oh.rearrange("p (c k) -> p c k", k=128)[:, lq : lq + nq, 0:D]
                rv = rcp[:, :nq].rearrange("p (c k) -> p c k", k=1)
                rv, ohv = bass.broadcast_tensor_aps(rv, ohv)
                nc.vector.tensor_mul(
                    osb.rearrange("p (c k) -> p c k", k=128)[
                        :, qc0 : qc0 + nq, D * h : D * h + D
                    ],
                    ohv,
                    rv,
                )
            for (si, c0, c1) in t["dmas"]:
                s0 = segments[si][0]
                osb = osb_tiles[si]
                L = segments[si][1] - s0
                r0, r1 = 128 * c0, min(128 * c1, L)
                nfc = (r1 - r0) // 128
                if nfc:
                    nc.sync.dma_start(
                        o_d[s0 + r0 : s0 + r0 + nfc * 128, :].rearrange(
                            "(c p) k -> p c k", p=128
                        ),
                        osb.rearrange("p (c k) -> p c k", k=128)[:, c0 : c0 + nfc, :],
                    )
                if (r1 - r0) % 128:
                    rr = r0 + nfc * 128
                    nc.sync.dma_start(
                        o_d[s0 + rr : s0 + r1, :],
                        osb[: r1 - rr, 128 * (c0 + nfc) : 128 * (c0 + nfc) + G * D],
                    )

        for g in range(len(tiles)):
            sp = ps_s.tile([128, TILE_W], f32, tag="sp", name=f"sp_{g}")
            emit_qk(g, sp)
            if g >= 1:
                emit_post(g - 1)
            se = sepool.tile([128, TILE_W], bf16, tag="se", name=f"se_{g}")
            se_tiles[g] = se
            sz = tiles[g]["sz"]
            nc.scalar.activation(
                se[:, :sz],
                sp[:, :sz],
                mybir.ActivationFunctionType.Exp,
                scale=SCALE,
            )
        emit_post(len(tiles) - 1)

    nc.compile()
    return nc


def _segments_from_cu(cu_seqlens, T):
    edges = sorted(set([0, T] + [int(c) for c in cu_seqlens if 0 < int(c) < T]))
    return [(edges[i], edges[i + 1]) for i in range(len(edges) - 1)]


def kernel(q, k, v, k_cache, v_cache, slot_mapping, cu_seqlens):
    global LAST_RESULT
    T = q.shape[0]
    nslots = k_cache.shape[0]

    # Emulate scatter-then-gather through the paged cache: for duplicate slots
    # the last writer wins, so token i reads back k[lastw[slot[i]]].
    slot = np.asarray(slot_mapping, dtype=np.int64)
    lastw = np.zeros(nslots, dtype=np.int64)
    lastw[slot] = np.arange(T)
    lw = lastw[slot]
    k_eff = np.asarray(k)[lw]
    v_eff = np.asarray(v)[lw]

    segments = _segments_from_cu(np.asarray(cu_seqlens), T)
    key = (T, tuple(segments))
    if key not in _PROGRAM_CACHE:
        _PROGRAM_CACHE[key] = _build_program(T, segments)
    nc = _PROGRAM_CACHE[key]

    bf = ml_dtypes.bfloat16
    qh = np.ascontiguousarray(
        np.asarray(q).reshape(T, NKV * G, D).transpose(1, 2, 0)
    ).astype(bf)  # [16, 64, T]
    kh = np.ascontiguousarray(k_eff.reshape(T, NKV, D).transpose(1, 2, 0)).astype(bf)
    vh = v_eff.reshape(T, NKV, D).astype(bf)  # [T, 8, 64]

    # vt: per-segment 128-row tiling of v rows, with a ones column at k=64
    nkt_tot = sum(ceil((s1 - s0) / 128) for (s0, s1) in segments)
    L0 = segments[0][1] - segments[0][0]
    use_w0 = L0 >= 1024

    in_maps = []
    for h in range(NKV):
        qT = np.ascontiguousarray(qh[2 * h : 2 * h + 2].reshape(128, T))
        kT = np.ascontiguousarray(kh[h])  # [64, T]
        vt = np.zeros((128, nkt_tot, 65), dtype=bf)
        kg = 0
        for (s0, s1) in segments:
            L = s1 - s0
            for kt in range(ceil(L / 128)):
                klo = s0 + 128 * kt
                kp = min(128, s1 - klo)
                vt[:kp, kg, :D] = vh[klo : klo + kp, h, :]
                vt[:, kg, D] = 1.0
                kg += 1
        m = {
            "qT": qT,
            "kT": kT,
            "vt": np.ascontiguousarray(vt.reshape(128, nkt_tot * 65)),
        }
        if use_w0:
            s00 = segments[0][0]
            w0 = np.concatenate(
                [kT[:, s00 : s00 + 128], qT[0:64, s00 + 128 : s00 + W0Q]], axis=1
            )
            m["w0"] = np.ascontiguousarray(w0)
        in_maps.append(m)

    res = run_bass_kernel_spmd(nc, in_maps, core_ids=list(range(8)), trace=TRACE)
    LAST_RESULT = res

    out = np.empty((T, NKV * G * D), dtype=np.float32)
    ov = out.reshape(T, NKV, G * D)
    for h in range(NKV):
        ov[:, h, :] = res.results[h]["o"]
    return out


# revision 44
# speedup vs baseline: 1.2510x; 1.0088x over previous
"""Paged-KV varlen causal GQA attention for Trainium2, sharded over 8 NeuronCores.

Problem (hardcoded from spec): T=4096 tokens, 16 q heads / 8 kv heads, head_dim=64,
scale=0.125. k/v are scattered into paged caches via slot_mapping, read back, and
causal varlen attention (segments from cu_seqlens) is computed.

Sharding: tensor-parallel over kv heads -- core h gets kv head h and its 2 GQA
query heads. slot_mapping / cu_seqlens handled on host (index math only).

Device kernel (per core): a single flat stream of score tiles [128, <=1536] f32
(2 PSUM buffers x 3 banks) packed with (head, key-tile) pieces:
  sp[keys, queries] = kT.T @ qT          (PE, per piece, bank-split)
  se = exp(0.125 * sp)                   (ScalarE, one exp per score tile -- the
                                          bottleneck engine; tiles are as wide as
                                          PSUM allows to amortize instr overhead)
  diagonal blocks: se *= trimask         (VectorE, one batched mul per block)
  o[q, 0:65] += se[:,qc].T @ [v | 1]     (PE with se as STATIONARY: output lands
                                          query-major, col 64 = softmax denom --
                                          no transposes / PSUM->SBUF copies)
  osb[q, d] = o[q, d] * 1/o[q, 64]       (VectorE reciprocal + broadcast mul)
"""

import os
from contextlib import ExitStack
from math import ceil

import numpy as np
import ml_dtypes

import concourse.bass as bass
import concourse.mybir as mybir
import concourse.tile as tile
from concourse import bacc
from concourse.bass_utils import run_bass_kernel_spmd

NKV = 8
G = 2
D = 64
SCALE = 0.125

TILE_W = 1024  # score tile width (2 PSUM banks of f32, triple-buffered)
BANK = 512     # f32 cols per PSUM bank
W0Q = 1024     # warmup tensor covers kT[0:128] + qT[128:1024] of segment 0

# test.py pokes these for profiling
TRACE = bool(int(os.environ.get("KERNEL_TRACE", "0")))
LAST_RESULT = None

_PROGRAM_CACHE = {}


def _plan(segments):
    """Pack the per-(segment, head) score streams into a flat list of score
    tiles. Returns (tiles, n_keytiles_total).

    Each tile dict:
      sz: used cols
      qk: [(h, s0, klo, kp, qa, qb, col, from_w0)]   QK subpieces (bank-split)
      masks: [(col, nfull)]                          batched trimask ranges
      pv: [(h, si, qc, col, cw, ktg, kp, start, stop)]
      norms: [(h, si, f, qc0, nq)]                   norm groups due after this tile
      dmas: [(si, c0, c1)]                           output row-blocks due
    """
    tiles = []
    cur = None
    chains = {}  # (h, si, qc) -> [ [tile_idx, col, cw, ktg, kp], ... ]
    cap_now = TILE_W

    def new_tile():
        nonlocal cur
        cur = {
            "sz": 0, "qk": [], "dve": [], "pmask": [],
            "pv": [], "norms": [], "dmas": [],
        }
        tiles.append(cur)

    new_tile()

    # global key-tile counter (vt layout is per-segment tiled on host)
    seg_ktg0 = []
    nkt_tot = 0
    for (s0, s1) in segments:
        seg_ktg0.append(nkt_tot)
        nkt_tot += ceil((s1 - s0) / 128)

    def place_span(h, si, kt, qa, qb, w0_ok):
        """Place one query span of key tile kt, splitting at tile caps and
        PSUM bank edges; register PV chunks."""
        nonlocal cur
        s0, s1 = segments[si]
        L = s1 - s0
        klo = 128 * kt
        kp = min(128, L - klo)
        ktg = seg_ktg0[si] + kt
        q = qa
        while q < qb:
            if cur["sz"] >= cap_now:
                new_tile()
            room = cap_now - cur["sz"]
            take = min(qb - q, room)
            col = cur["sz"]
            # QK subpieces: split at bank edges and at the w0 boundary
            qq = q
            while qq < q + take:
                c = col + (qq - q)
                lim = qq + min(q + take - qq, BANK - (c % BANK))
                use_w0 = w0_ok and 128 <= qq < W0Q
                if use_w0 and lim > W0Q:
                    lim = W0Q
                cur["qk"].append((h, s0, klo, kp, qq, lim, c, use_w0))
                qq = lim
            # PV chunks (128-query aligned within the segment)
            for qs in range(q, q + take, 128):
                qc = qs // 128
                cw = min(128, qb - qs, q + take - qs)
                chains.setdefault((h, si, qc), []).append(
                    [len(tiles) - 1, col + (qs - q), cw, ktg, kp]
                )
            cur["sz"] += take
            if cur["sz"] % 128:  # ragged tail: re-align for bank safety
                cur["sz"] += 128 - cur["sz"] % 128
            q += take

    def place_diag_run(h, si, kts, w0_ok, act_first):
        """Place diagonal chunks contiguously. Chunk kt0 (query chunk 0 -- its
        whole softmax comes from this block, so it needs exact exp) goes to the
        ACT path; the rest are computed by DVE (Schraudolph exp2 bit trick)
        with the causal mask fused into the Pool-engine multiply. act_first
        puts the ACT chunk on the side that abuts the tile's other ACT cols."""
        nonlocal cur
        s0, s1 = segments[si]
        L = s1 - s0
        total = sum(min(128, L - 128 * kt) for kt in kts)
        if cur["sz"] + total > cap_now and cur["sz"] > 0:
            new_tile()
        kts_act = [kt for kt in kts if kt == 0]
        kts_dve = [kt for kt in kts if kt != 0]
        order = kts_act + kts_dve if act_first else kts_dve + kts_act
        dve_col0 = cur["sz"] + (128 * len(kts_act) if act_first else 0)
        for kt in order:
            klo = 128 * kt
            kp = min(128, L - klo)
            place_span(h, si, kt, klo, klo + kp, w0_ok)
        if kts_dve:
            cur["dve"].append((dve_col0, len(kts_dve), False))
        for i, kt in enumerate(kts_act):
            base = cur["sz"] - 128 * len(kts_act) if not act_first else dve_col0 - 128 * len(kts_act)
            cur["pmask"].append((base + 128 * i, False))

    def place_diag_chunks(h, si, kts, act, w0_ok=False, fast_mask=False):
        """Place diagonal chunks at the current position (caller guarantees
        fit): one dve range (or pmask entries if act=True for the kt0 chunk).
        fast_mask runs the mask multiply on DVE instead of Pool -- used for
        the closing tiles where Pool's serial queue would sit on the tail."""
        s0, s1 = segments[si]
        L = s1 - s0
        col0 = cur["sz"]
        for kt in kts:
            klo = 128 * kt
            place_span(h, si, kt, klo, klo + min(128, L - klo), w0_ok)
        if act:
            for i in range(len(kts)):
                cur["pmask"].append((col0 + 128 * i, fast_mask))
        else:
            cur["dve"].append((col0, len(kts), fast_mask))

    def take_nd(h, si, ndq, ncols, w0_ok=False):
        """Consume exactly ncols of non-diagonal pieces from the queue into
        the current tile (caller guarantees fit)."""
        left = ncols
        while left > 0:
            kt, qa, qb = ndq[0]
            take = min(qb - qa, left)
            place_span(h, si, kt, qa, qa + take, w0_ok)
            if qa + take == qb:
                ndq.pop(0)
            else:
                ndq[0][1] = qa + take
            left -= take

    nseg = len(segments)
    for si, (s0, s1) in enumerate(segments):
        L = s1 - s0
        nkt = ceil(L / 128)
        nqc = ceil(L / 128)
        for h in range(G):
            first_sh = si == 0 and h == 0 and L >= 1024
            last_sh = (
                si == nseg - 1 and h == G - 1 and nqc >= 8 and L % 128 == 0
            )
            nd = [kt for kt in range(nkt - 1) if 128 * (kt + 1) < L]
            # Fast layouts for 1024-token segments keep every ScalarE exp wide
            # enough (>= ~1024 cols) to cover the sp double-buffer turnaround
            # (QK of tile g can only start once exp of tile g-2 freed the
            # PSUM slot), by splitting the diagonal run across two tiles.
            fast = L == 1024 and nkt == 8
            if first_sh and fast:
                ndq = [[kt, 128 * (kt + 1), L] for kt in nd]
                cap_now = TILE_W
                take_nd(h, si, ndq, 384, True)
                new_tile()
                take_nd(h, si, ndq, 512, True)
                new_tile()
                take_nd(h, si, ndq, 512, True)
                place_diag_chunks(h, si, [1, 2, 3, 4], False, True)
                new_tile()
                place_diag_chunks(h, si, [5, 6, 7], False, True)
                place_diag_chunks(h, si, [0], True)
                take_nd(h, si, ndq, 512, True)
                new_tile()
                take_nd(h, si, ndq, 1024, True)
                new_tile()
                take_nd(h, si, ndq, 640, True)
                new_tile()
            elif last_sh and fast:
                # tail: staggered bank closure -- qc0..2 close first, then
                # qc3..6, then qc7 alone in a small final tile, so norms and
                # output DMAs pipeline instead of serializing at the end
                ndqa = [[kt, 128 * (kt + 1), L - 128] for kt in nd]
                ndqb = [[kt, L - 128, L] for kt in nd]
                take_nd(h, si, ndqa, 1024)
                new_tile()
                take_nd(h, si, ndqa, 1024)
                new_tile()
                take_nd(h, si, ndqa, 640)
                place_diag_chunks(h, si, [0], True, fast_mask=True)
                place_diag_chunks(h, si, [1, 2], False, fast_mask=True)
                new_tile()
                place_diag_chunks(h, si, [3, 4, 5, 6], False, fast_mask=True)
                take_nd(h, si, ndqb, 512)
                new_tile()
                take_nd(h, si, ndqb, 384)
                place_diag_chunks(h, si, [7], False, fast_mask=True)
                new_tile()
            elif fast:
                ndq = [[kt, 128 * (kt + 1), L] for kt in nd]
                take_nd(h, si, ndq, 1024)
                new_tile()
                take_nd(h, si, ndq, 512)
                place_diag_chunks(h, si, [1, 2, 3, 4], False)
                new_tile()
                place_diag_chunks(h, si, [5, 6, 7], False)
                place_diag_chunks(h, si, [0], True)
                take_nd(h, si, ndq, 512)
                new_tile()
                take_nd(h, si, ndq, 1024)
                new_tile()
                take_nd(h, si, ndq, 512)
                new_tile()
            elif first_sh:
                # warmup: small leading non-diag tiles, diag last
                cap_now = 384
                place_span(h, si, 0, 128, min(512, L), True)
                new_tile()
                cap_now = 512
                place_span(h, si, 0, 512, min(1024, L), True)
                new_tile()
                cap_now = TILE_W
                if L > 1024:
                    place_span(h, si, 0, 1024, L, True)
                for kt in nd[1:]:
                    place_span(h, si, kt, 128 * (kt + 1), L, True)
                place_diag_run(h, si, range(nkt), True, act_first=True)
            elif last_sh:
                # tail: isolate the last 128 queries' pieces in small final
                # tiles so the closing exp->PV->norm->DMA chain is short
                for kt in nd:
                    place_span(h, si, kt, 128 * (kt + 1), L - 128, False)
                place_diag_run(h, si, range(nkt - 1), False, act_first=False)
                for kt in nd[:-2]:
                    place_span(h, si, kt, L - 128, L, False)
                if cur["sz"] > 0:
                    new_tile()
                for kt in nd[-2:]:
                    place_span(h, si, kt, L - 128, L, False)
                place_diag_run(h, si, [nkt - 1], False, act_first=False)
            else:
                for kt in nd:
                    place_span(h, si, kt, 128 * (kt + 1), L, False)
                place_diag_run(h, si, range(nkt), False, act_first=True)

    while tiles and tiles[-1]["sz"] == 0:
        tiles.pop()

    # qchunk -> oh-bank mapping: 4 chunks per PSUM bank, except the last
    # segment where the final chunk gets its own bank so its accumulation
    # group closes (and is normalized/DMAd) independently of chunks 4..nqc-2
    def oh_banks(h, si):
        # the tail-isolating 3-bank split only works for the last head, whose
        # layout defers all qc7 pieces to the end; other streams accumulate
        # qc7 from the start so all banks are concurrently live (only 2 fit)
        L = segments[si][1] - segments[si][0]
        nqc = ceil(L / 128)
        if si == nseg - 1 and h == G - 1 and nqc == 8:
            return [[0, 1, 2], [3, 4, 5, 6], [7]]
        return [
            list(range(4 * f, min(4 * f + 4, nqc))) for f in range(ceil(nqc / 4))
        ]

    qc_bank = {}  # (h, si, qc) -> (f, fbase)
    for si in range(nseg):
        for h in range(G):
            for f, qcs in enumerate(oh_banks(h, si)):
                for qc in qcs:
                    qc_bank[(h, si, qc)] = (f, qcs[0])

    # start/stop flags are PER PSUM BANK (zero region), not per qchunk chain:
    # start_tensor_calc pends-zero the whole 2KB bank, so only the
    # chronologically first matmul into an oh tile may set it, and only the
    # last sets stop. Per-byte lazy zeroing makes each chain's first write a
    # plain store and later writes accumulates, regardless of interleaving.
    oh_groups = {}
    for (h, si, qc), ch in chains.items():
        for e in ch:
            oh_groups.setdefault((h, si, qc_bank[(h, si, qc)][0]), []).append(e)
    for grp in oh_groups.values():
        grp.sort(key=lambda e: (e[0], e[1]))
        for e in grp:
            e.append(e is grp[0])
            e.append(e is grp[-1])
    for (h, si, qc), ch in chains.items():
        for e in ch:
            tiles[e[0]]["pv"].append((h, si, qc, e[1], e[2], e[3], e[4], e[5], e[6]))

    # order pv entries in each tile by emission col to keep per-chain order
    for t in tiles:
        t["pv"].sort(key=lambda p: (p[3], p[0], p[1], p[2]))

    # norm groups (one per oh bank): due after max last tile over the chains
    for si in range(nseg):
        norm_due = {}  # (h, qc) -> tile where that qchunk's norm is emitted
        for h in range(G):
            for f, gq in enumerate(oh_banks(h, si)):
                last = max(chains[(h, si, qc)][-1][0] for qc in gq)
                tiles[last]["norms"].append((h, si, f, gq[0], len(gq)))
                for qc in gq:
                    norm_due[(h, qc)] = last
        # output DMAs: row groups follow the last head's bank split
        row_groups = oh_banks(G - 1, si)
        if len(row_groups) > 2:
            for gq in row_groups:
                due = max(norm_due[(h, qc)] for h in range(G) for qc in gq)
                tiles[due]["dmas"].append((si, gq[0], gq[-1] + 1))
        else:
            due = max(norm_due.values())
            nqc = ceil((segments[si][1] - segments[si][0]) / 128)
            tiles[due]["dmas"].append((si, 0, nqc))

    return tiles, nkt_tot, qc_bank


def _build_program(T, segments):
    f32 = mybir.dt.float32
    bf16 = mybir.dt.bfloat16

    tiles, nkt_tot, qc_bank = _plan(segments)
    L0 = segments[0][1] - segments[0][0]
    use_w0 = L0 >= 1024

    nc = bacc.Bacc(
        "TRN2",
        target_bir_lowering=False,
        debug=False,
        enable_asserts=False,
        num_devices=8,
    )
    qT_d = nc.dram_tensor("qT", [128, T], bf16, kind="ExternalInput").ap()
    kT_d = nc.dram_tensor("kT", [64, T], bf16, kind="ExternalInput").ap()
    vt_d = nc.dram_tensor("vt", [128, nkt_tot * 65], bf16, kind="ExternalInput").ap()
    if use_w0:
        w0_d = nc.dram_tensor("w0", [64, W0Q], bf16, kind="ExternalInput").ap()
    o_d = nc.dram_tensor("o", [T, G * D], f32, kind="ExternalOutput").ap()

    with tile.TileContext(nc) as tc, ExitStack() as ctx:
        const = ctx.enter_context(tc.tile_pool(name="const", bufs=1))
        inpool = ctx.enter_context(tc.tile_pool(name="inp", bufs=1))
        sepool = ctx.enter_context(tc.tile_pool(name="se", bufs=6))
        opool = ctx.enter_context(tc.tile_pool(name="osb", bufs=2))
        rpool = ctx.enter_context(tc.tile_pool(name="rcp", bufs=4))
        ipool = ctx.enter_context(tc.tile_pool(name="i32", bufs=3))
        ps_s = ctx.enter_context(tc.tile_pool(name="ps_s", bufs=3, space="PSUM"))
        ps_o = ctx.enter_context(tc.tile_pool(name="ps_o", bufs=2, space="PSUM"))

        # trimask[p, c] = 1 if c >= p else 0 (valid = query col >= key partition)
        trimask = const.tile([128, 128], bf16)
        nc.gpsimd.memset(trimask, 0.0)
        nc.gpsimd.affine_select(
            out=trimask,
            in_=trimask,
            compare_op=mybir.AluOpType.is_gt,
            fill=1.0,
            base=0,
            pattern=[[-1, 128]],
            channel_multiplier=1,
        )

        qT = inpool.tile([128, T], bf16)
        kT = inpool.tile([128, T], bf16)
        vt = inpool.tile([128, nkt_tot * 65], bf16)
        vtv = vt.rearrange("p (n k) -> p n k", k=65)
        if use_w0:
            w0 = inpool.tile([64, W0Q], bf16, padded_shape=[128, W0Q])

        # ---- input DMAs: all issued up front ----
        # kT halves are loaded separately (h0 rows 0:64 first) so head 0's QK
        # stream unblocks as early as possible; w0 covers the whole first
        # (seg0, h0) non-diagonal stream so tiles 0..3 only depend on 1-2 DMAs
        s00, s01 = segments[0]
        kg = 0
        if use_w0:
            nc.sync.dma_start(w0, w0_d)
        first = True
        for (s0, s1) in segments:
            nkt = ceil((s1 - s0) / 128)
            nc.sync.dma_start(kT[0:64, s0:s1], kT_d[:, s0:s1])
            if first and use_w0:
                nc.sync.dma_start(
                    vt[:, kg * 65 : (kg + nkt) * 65],
                    vt_d[:, kg * 65 : (kg + nkt) * 65],
                )
                nc.sync.dma_start(qT[:, s0:s1], qT_d[:, s0:s1])
            else:
                nc.sync.dma_start(qT[:, s0:s1], qT_d[:, s0:s1])
                nc.sync.dma_start(
                    vt[:, kg * 65 : (kg + nkt) * 65],
                    vt_d[:, kg * 65 : (kg + nkt) * 65],
                )
            nc.sync.dma_start(kT[64:128, s0:s1], kT_d[:, s0:s1])
            kg += nkt
            first = False

        # ---- main stream ----
        se_tiles = {}
        oh_tiles = {}
        osb_tiles = {}

        def get_oh(h, si, f):
            key = (h, si, f)
            if key not in oh_tiles:
                oh_tiles[key] = ps_o.tile(
                    [128, 512], f32, tag="oh", name=f"oh_{h}_{si}_{f}"
                )
            return oh_tiles[key]

        def get_osb(si):
            if si not in osb_tiles:
                nqc = ceil((segments[si][1] - segments[si][0]) / 128)
                osb_tiles[si] = opool.tile(
                    [128, nqc * 128], f32, tag="osb", name=f"osb_{si}"
                )
            return osb_tiles[si]

        def emit_qk(g, sp):
            # diagonal subpieces first: the DVE exp2 pass only needs those,
            # so it can start while the rest of the tile's QKs still run
            dcols = set()
            for (c0, n, _fm) in tiles[g]["dve"]:
                dcols.update(range(c0, c0 + 128 * n, 128))
            tiles[g]["qk"].sort(key=lambda p: (p[6] not in dcols, p[6]))
            for (h, s0, klo, kp, qa, qb, col, from_w0) in tiles[g]["qk"]:
                lhsT = (
                    w0[0:64, 0:kp]
                    if (from_w0 and klo == 0)
                    else kT[64 * h : 64 * h + 64, s0 + klo : s0 + klo + kp]
                )
                rhs = (
                    w0[0:64, qa:qb]
                    if from_w0
                    else qT[64 * h : 64 * h + 64, s0 + qa : s0 + qb]
                )
                nc.tensor.matmul(
                    sp[:kp, col : col + qb - qa],
                    lhsT,
                    rhs,
                    start=True,
                    stop=True,
                    tile_position=(64 * h, 0),
                )

        def emit_post(g):
            t = tiles[g]
            se = se_tiles.pop(g)
            for (h, si, qc, col, cw, ktg, kp, start, stop) in t["pv"]:
                f, fbase = qc_bank[(h, si, qc)]
                oh = get_oh(h, si, f)
                lq = qc - fbase
                nc.tensor.matmul(
                    oh[:cw, 128 * lq : 128 * lq + 65],
                    se[:kp, col : col + cw],
                    vtv[:kp, ktg, :],
                    start=start,
                    stop=stop,
                )
            for (h, si, f, qc0, nq) in t["norms"]:
                oh = oh_tiles[(h, si, f)]
                osb = get_osb(si)
                lq = 0  # bank-local offset: each norm group is a whole oh tile
                uniq = f"{h}_{si}_{qc0}"
                rcp = rpool.tile([128, 8], f32, tag="rcp", name=f"rcp_{uniq}")
                nc.vector.reciprocal(
                    rcp[:, :nq],
                    oh[:, 128 * lq + 64 : 128 * (lq + nq - 1) + 65 : 128],
                )
                ohv = oh.rearrange("p (c k) -> p c k", k=128)[:, lq : lq + nq, 0:D]
                rv = rcp[:, :nq].rearrange("p (c k) -> p c k", k=1)
                rv, ohv = bass.broadcast_tensor_aps(rv, ohv)
                nc.vector.tensor_mul(
                    osb.rearrange("p (c k) -> p c k", k=128)[
                        :, qc0 : qc0 + nq, D * h : D * h + D
                    ],
                    ohv,
                    rv,
                )
            for (si, c0, c1) in t["dmas"]:
                s0 = segments[si][0]
                osb = osb_tiles[si]
                L = segments[si][1] - s0
                r0, r1 = 128 * c0, min(128 * c1, L)
                nfc = (r1 - r0) // 128
                if nfc:
                    nc.sync.dma_start(
                        o_d[s0 + r0 : s0 + r0 + nfc * 128, :].rearrange(
                            "(c p) k -> p c k", p=128
                        ),
                        osb.rearrange("p (c k) -> p c k", k=128)[:, c0 : c0 + nfc, :],
                    )
                if (r1 - r0) % 128:
                    rr = r0 + nfc * 128
                    nc.sync.dma_start(
                        o_d[s0 + rr : s0 + r1, :],
                        osb[: r1 - rr, 128 * (c0 + nfc) : 128 * (c0 + nfc) + G * D],
                    )

        # Schraudolph exp2: exp(SCALE*s) = 2^(SCALE*s*log2e) approximated by
        # int32 bit assembly: i = round(t*2^23) + (127<<23) - C, bitcast f32.
        # ~3% max weight error, used only on diagonal blocks for query chunks
        # >= 1 (diluted by exact non-diag weights; qchunk 0 stays on ScalarE).
        LOG2E = 1.4426950408889634
        A_TS = float(SCALE * LOG2E * (1 << 23))
        B_TS = float((127 << 23) - 361400)
        i32dt = mybir.dt.int32

        for g in range(len(tiles)):
            t = tiles[g]
            sz = t["sz"]
            sp = ps_s.tile([128, TILE_W], f32, tag="sp", name=f"sp_{g}")
            emit_qk(g, sp)
            se = sepool.tile([128, TILE_W], bf16, tag="se", name=f"se_{g}")
            se_tiles[g] = se
            # DVE bit-trick exp2 + Pool fused-mask multiply
            for ri, (c0, n, fm) in enumerate(t["dve"]):
                i32 = ipool.tile(
                    [128, 512], i32dt, tag="i32", name=f"i32_{g}_{ri}", bufs=3
                )
                a = c0
                b = c0 + 128 * n
                io = i32[:, : 128 * n]
                nc.vector.tensor_scalar(
                    io,
                    sp[:, a:b],
                    A_TS,
                    B_TS,
                    mybir.AluOpType.mult,
                    mybir.AluOpType.add,
                )
                sev = se[:, a:b].rearrange("p (c k) -> p c k", k=128)
                iv = io.bitcast(f32).rearrange("p (c k) -> p c k", k=128)
                triv = trimask.rearrange("p (c k) -> p c k", k=128)
                triv, iv = bass.broadcast_tensor_aps(triv, iv)
                eng = nc.vector if fm else nc.gpsimd
                eng.tensor_mul(sev, iv, triv)
            if g >= 2:
                emit_post(g - 2)
            # ScalarE exp over the complement of the DVE ranges
            pos = 0
            acts = []
            for (c0, n, _fm) in sorted(t["dve"]):
                if c0 > pos:
                    acts.append((pos, c0))
                pos = c0 + 128 * n
            if pos < sz:
                acts.append((pos, sz))
            for (a, b) in acts:
                nc.scalar.activation(
                    se[:, a:b],
                    sp[:, a:b],
                    mybir.ActivationFunctionType.Exp,
                    scale=SCALE,
                )
            # Pool mask for ScalarE-path diagonal chunks (query chunk 0)
            for (c, fm) in t["pmask"]:
                eng = nc.vector if fm else nc.gpsimd
                eng.tensor_mul(se[:, c : c + 128], se[:, c : c + 128], trimask)
        emit_post(len(tiles) - 2)
        emit_post(len(tiles) - 1)

    nc.compile()
    return nc


def _segments_from_cu(cu_seqlens, T):
    edges = sorted(set([0, T] + [int(c) for c in cu_seqlens if 0 < int(c) < T]))
    return [(edges[i], edges[i + 1]) for i in range(len(edges) - 1)]


def kernel(q, k, v, k_cache, v_cache, slot_mapping, cu_seqlens):
    global LAST_RESULT
    T = q.shape[0]
    nslots = k_cache.shape[0]

    # Emulate scatter-then-gather through the paged cache: for duplicate slots
    # the last writer wins, so token i reads back k[lastw[slot[i]]].
    slot = np.asarray(slot_mapping, dtype=np.int64)
    lastw = np.zeros(nslots, dtype=np.int64)
    lastw[slot] = np.arange(T)
    lw = lastw[slot]
    k_eff = np.asarray(k)[lw]
    v_eff = np.asarray(v)[lw]

    segments = _segments_from_cu(np.asarray(cu_seqlens), T)
    key = (T, tuple(segments))
    if key not in _PROGRAM_CACHE:
        _PROGRAM_CACHE[key] = _build_program(T, segments)
    nc = _PROGRAM_CACHE[key]

    bf = ml_dtypes.bfloat16
    qh = np.ascontiguousarray(
        np.asarray(q).reshape(T, NKV * G, D).transpose(1, 2, 0)
    ).astype(bf)  # [16, 64, T]
    kh = np.ascontiguousarray(k_eff.reshape(T, NKV, D).transpose(1, 2, 0)).astype(bf)
    vh = v_eff.reshape(T, NKV, D).astype(bf)  # [T, 8, 64]

    # vt: per-segment 128-row tiling of v rows, with a ones column at k=64
    nkt_tot = sum(ceil((s1 - s0) / 128) for (s0, s1) in segments)
    L0 = segments[0][1] - segments[0][0]
    use_w0 = L0 >= 1024

    in_maps = []
    for h in range(NKV):
        qT = np.ascontiguousarray(qh[2 * h : 2 * h + 2].reshape(128, T))
        kT = np.ascontiguousarray(kh[h])  # [64, T]
        vt = np.zeros((128, nkt_tot, 65), dtype=bf)
        kg = 0
        for (s0, s1) in segments:
            L = s1 - s0
            for kt in range(ceil(L / 128)):
                klo = s0 + 128 * kt
                kp = min(128, s1 - klo)
                vt[:kp, kg, :D] = vh[klo : klo + kp, h, :]
                vt[:, kg, D] = 1.0
                kg += 1
        m = {
            "qT": qT,
            "kT": kT,
            "vt": np.ascontiguousarray(vt.reshape(128, nkt_tot * 65)),
        }
        if use_w0:
            s00 = segments[0][0]
            w0 = np.concatenate(
                [kT[:, s00 : s00 + 128], qT[0:64, s00 + 128 : s00 + W0Q]], axis=1
            )
            m["w0"] = np.ascontiguousarray(w0)
        in_maps.append(m)

    res = run_bass_kernel_spmd(nc, in_maps, core_ids=list(range(8)), trace=TRACE)
    LAST_RESULT = res

    out = np.empty((T, NKV * G * D), dtype=np.float32)
    ov = out.reshape(T, NKV, G * D)
    for h in range(NKV):
        ov[:, h, :] = res.results[h]["o"]
    return out


# revision 53
# speedup vs baseline: 1.4040x; 1.1223x over previous
"""Paged-KV varlen causal GQA attention for Trainium2, sharded over 8 NeuronCores.

Problem (hardcoded from spec): T=4096 tokens, 16 q heads / 8 kv heads, head_dim=64,
scale=0.125. k/v are scattered into paged caches via slot_mapping, read back, and
causal varlen attention (segments from cu_seqlens) is computed.

Sharding: tensor-parallel over kv heads -- core h gets kv head h and its 2 GQA
query heads. slot_mapping / cu_seqlens handled on host (index math only).

Device kernel (per core): a single flat stream of score tiles [128, <=1536] f32
(2 PSUM buffers x 3 banks) packed with (head, key-tile) pieces:
  sp[keys, queries] = kT.T @ qT          (PE, per piece, bank-split)
  se = exp(0.125 * sp)                   (ScalarE, one exp per score tile -- the
                                          bottleneck engine; tiles are as wide as
                                          PSUM allows to amortize instr overhead)
  diagonal blocks: se *= trimask         (VectorE, one batched mul per block)
  o[q, 0:65] += se[:,qc].T @ [v | 1]     (PE with se as STATIONARY: output lands
                                          query-major, col 64 = softmax denom --
                                          no transposes / PSUM->SBUF copies)
  osb[q, d] = o[q, d] * 1/o[q, 64]       (VectorE reciprocal + broadcast mul)
"""

import os
from contextlib import ExitStack
from math import ceil

import numpy as np
import ml_dtypes

import concourse.bass as bass
import concourse.mybir as mybir
import concourse.tile as tile
from concourse import bacc
from concourse.bass_utils import run_bass_kernel_spmd

NKV = 8
G = 2
D = 64
SCALE = 0.125

TILE_A = 1024  # ScalarE score tile width (2 PSUM banks of f32, 2 buffers)
TILE_D = 512   # DVE exp2 score tile width (1 PSUM bank, 2 buffers)
BANK = 512     # f32 cols per PSUM bank
W0Q = 1024     # warmup tensor covers kT[0:128] + qT[0:1024] of segment 0

# test.py pokes these for profiling
TRACE = bool(int(os.environ.get("KERNEL_TRACE", "0")))
LAST_RESULT = None

_PROGRAM_CACHE = {}


def _plan(segments):
    """Pack the per-(segment, head) score work into TWO independent tile
    streams, interleaved in one creation-order list:
      kind 'a': ScalarE-exp tiles, [128, <=1024] f32 (2 PSUM banks x 2 bufs)
      kind 'd': DVE-exp2 tiles,    [128, <=512]  f32 (1 PSUM bank  x 2 bufs)
    Each engine paces its own stream, so an sp slot is freed by exactly one
    reader and the pipelines decouple.

    Tile dicts: kind, sz, qk, ts, masks, pmask, pv, norms, dmas.
    """
    tiles = []
    cur = {"a": None, "d": None}
    acap = [384, 512]  # warmup caps for the first A tiles, then TILE_A
    chains = {}  # (h, si, qc) -> [ [tile_idx, col, cw, ktg, kp], ... ]

    def new_tile(kind):
        t = {
            "kind": kind, "idx": len(tiles), "sz": 0, "qk": [], "ts": [],
            "masks": [], "pmask": [], "pv": [], "norms": [], "dmas": [],
        }
        tiles.append(t)
        cur[kind] = t
        return t

    def close(kind):
        cur[kind] = None

    def cap(kind):
        if kind == "d":
            return TILE_D
        na = sum(1 for t in tiles if t["kind"] == "a")
        if cur["a"] is not None:
            na -= 1
        return acap[na] if na < len(acap) else TILE_A

    # global key-tile counter (vt layout is per-segment tiled on host)
    seg_ktg0 = []
    nkt_tot = 0
    for (s0, s1) in segments:
        seg_ktg0.append(nkt_tot)
        nkt_tot += ceil((s1 - s0) / 128)

    def place_span(kind, h, si, kt, qa, qb, w0_ok):
        """Place one query span of key tile kt into the `kind` stream,
        splitting at tile caps and PSUM bank edges; register PV chunks.
        Returns [(tile, col, ncols)] placements."""
        s0, s1 = segments[si]
        L = s1 - s0
        klo = 128 * kt
        kp = min(128, L - klo)
        ktg = seg_ktg0[si] + kt
        placements = []
        q = qa
        while q < qb:
            if cur[kind] is None or cur[kind]["sz"] >= cap(kind):
                new_tile(kind)
            t = cur[kind]
            room = cap(kind) - t["sz"]
            take = min(qb - q, room)
            col = t["sz"]
            placements.append((t, col, take))
            qq = q
            while qq < q + take:
                c = col + (qq - q)
                lim = qq + min(q + take - qq, BANK - (c % BANK))
                use_w0 = w0_ok and qq < W0Q
                if use_w0 and lim > W0Q:
                    lim = W0Q
                t["qk"].append((h, s0, klo, kp, qq, lim, c, use_w0))
                qq = lim
            for qs in range(q, q + take, 128):
                qc = qs // 128
                cw = min(128, qb - qs, q + take - qs)
                chains.setdefault((h, si, qc), []).append(
                    [t["idx"], col + (qs - q), cw, ktg, kp]
                )
            t["sz"] += take
            if t["sz"] % 128:
                t["sz"] += 128 - t["sz"] % 128
            q += take
        return placements

    def place_diag(h, si, kts, act, w0_ok=False, fast_mask=False):
        """Diagonal chunks: kt0 (act=True) goes on the A stream with a
        mask-multiply (pmask); others go on the D stream (DVE exp2) with
        the causal mask applied by Pool (or DVE when fast_mask)."""
        s0, s1 = segments[si]
        L = s1 - s0
        for kt in kts:
            klo = 128 * kt
            kp = min(128, L - klo)
            pl = place_span("a" if act else "d", h, si, kt, klo, klo + kp, w0_ok)
            for (t, col, n) in pl:
                if act:
                    t["pmask"].append((col, fast_mask))
                else:
                    t["ts"].append((col, n))
                    t["masks"].append((col, n // 128, fast_mask))

    def take_nd(kind, h, si, ndq, ncols, w0_ok=False):
        """Consume ncols of non-diagonal pieces into the given stream; D-
        stream cols get ts entries (DVE exp2, no mask needed)."""
        left = ncols
        while left > 0:
            kt, qa, qb = ndq[0]
            take = min(qb - qa, left)
            pl = place_span(kind, h, si, kt, qa, qa + take, w0_ok)
            if kind == "d":
                for (t, col, n) in pl:
                    t["ts"].append((col, n))
            if qa + take == qb:
                ndq.pop(0)
            else:
                ndq[0][1] = qa + take
            left -= take

    nseg = len(segments)
    for si, (s0, s1) in enumerate(segments):
        L = s1 - s0
        nkt = ceil(L / 128)
        nqc = ceil(L / 128)
        for h in range(G):
            first_sh = si == 0 and h == 0 and L >= 1024
            last_sh = (
                si == nseg - 1 and h == G - 1 and nqc >= 8 and L % 128 == 0
            )
            nd = [kt for kt in range(nkt - 1) if 128 * (kt + 1) < L]
            fast = L == 1024 and nkt == 8
            if fast and not last_sh:
                w0_ok = first_sh
                ndq = [[kt, 128 * (kt + 1), L] for kt in nd]
                # route the last 640 nd cols (late stripes of kt4..6, well
                # diluted query chunks) through the DVE stream; with the kt0
                # diagonal on the A stream both streams then hold an exact
                # number of tiles per (seg, head), which keeps oh-bank norms
                # strictly before the next tenant's first PV in program order
                extras = [[4, 768, 1024], [5, 768, 1024], [6, 896, 1024]]
                ndq[4][2] = 768
                ndq[5][2] = 768
                ndq = [p for p in ndq if p[1] < p[2] and p[0] != 6]
                take_nd("a", h, si, ndq, 896, w0_ok)
                place_diag(h, si, [0], True, w0_ok)
                place_diag(h, si, [1, 2, 3, 4], False, w0_ok)
                take_nd("a", h, si, ndq, 1024, w0_ok)
                place_diag(h, si, [5, 6, 7], False, w0_ok)
                take_nd("d", h, si, extras, 640, w0_ok)
                take_nd("a", h, si, ndq, 1024, w0_ok)
                close("a")
                close("d")
            elif fast and last_sh:
                # tail: qchunk 7's pieces and the final diagonal chunk come
                # last, in their own small tiles, for a short closing chain
                ndqa = [[kt, 128 * (kt + 1), L - 128] for kt in nd]
                ndqb = [[kt, L - 128, L] for kt in nd]
                extras = [[4, 640, 896], [5, 768, 896]]
                ndqa[4][2] = 640
                ndqa[5] = [5, 768, 768]
                ndqa = [p for p in ndqa if p[1] < p[2]]
                take_nd("a", h, si, ndqa, 1024)
                place_diag(h, si, [1, 2, 3], False, fast_mask=True)
                take_nd("a", h, si, ndqa, 1024)
                place_diag(h, si, [4, 5, 6], False, fast_mask=True)
                take_nd("d", h, si, extras, 384)
                take_nd("a", h, si, ndqa, 256)
                place_diag(h, si, [0], True, fast_mask=True)
                close("a")
                take_nd("a", h, si, ndqb, 896)
                close("a")
                close("d")
                place_diag(h, si, [7], False, fast_mask=True)
                close("d")
            else:
                # generic fallback: everything on the A stream, diag last,
                # masks on Pool
                for kt in nd:
                    place_span("a", h, si, kt, 128 * (kt + 1), L, first_sh)
                place_diag(h, si, [0], True, first_sh)
                place_diag(h, si, list(range(1, nkt)), False, first_sh)

    # coalesce adjacent ts / mask ranges per tile
    for t in tiles:
        for key in ("ts",):
            t[key].sort()
            merged = []
            for (c0, w) in t[key]:
                if merged and merged[-1][0] + merged[-1][1] == c0:
                    merged[-1][1] += w
                else:
                    merged.append([c0, w])
            t[key] = [tuple(m) for m in merged]
        t["masks"].sort()
        merged = []
        for (c0, n, fm) in t["masks"]:
            if merged and merged[-1][0] + 128 * merged[-1][1] == c0 and merged[-1][2] == fm:
                merged[-1][1] += n
            else:
                merged.append([c0, n, fm])
        t["masks"] = [tuple(m) for m in merged]

    # qchunk -> oh-bank mapping: 4 chunks per PSUM bank, except the last
    # (segment, head) where the final chunk gets its own bank so its
    # accumulation group closes independently
    def oh_banks(h, si):
        L = segments[si][1] - segments[si][0]
        nqc = ceil(L / 128)
        if si == nseg - 1 and h == G - 1 and nqc == 8:
            return [[0, 1, 2], [3, 4, 5, 6], [7]]
        return [
            list(range(4 * f, min(4 * f + 4, nqc))) for f in range(ceil(nqc / 4))
        ]

    qc_bank = {}  # (h, si, qc) -> (f, fbase)
    for si in range(nseg):
        for h in range(G):
            for f, qcs in enumerate(oh_banks(h, si)):
                for qc in qcs:
                    qc_bank[(h, si, qc)] = (f, qcs[0])

    # start/stop flags are PER PSUM BANK (zero region), not per qchunk chain:
    # start_tensor_calc pends-zero the whole 2KB bank, so only the
    # chronologically first matmul into an oh tile may set it, and only the
    # last sets stop. Per-byte lazy zeroing makes each chain's first write a
    # plain store and later writes accumulates, regardless of interleaving.
    oh_groups = {}
    for (h, si, qc), ch in chains.items():
        for e in ch:
            oh_groups.setdefault((h, si, qc_bank[(h, si, qc)][0]), []).append(e)
    for grp in oh_groups.values():
        grp.sort(key=lambda e: (e[0], e[1]))
        for e in grp:
            e.append(e is grp[0])
            e.append(e is grp[-1])
    for (h, si, qc), ch in chains.items():
        for e in ch:
            tiles[e[0]]["pv"].append((h, si, qc, e[1], e[2], e[3], e[4], e[5], e[6]))

    # order pv entries in each tile by emission col to keep per-chain order
    for t in tiles:
        t["pv"].sort(key=lambda p: (p[3], p[0], p[1], p[2]))

    # norm groups (one per oh bank): due after max last tile over the chains
    for si in range(nseg):
        norm_due = {}
        for h in range(G):
            for f, gq in enumerate(oh_banks(h, si)):
                last = max(chains[(h, si, qc)][-1][0] for qc in gq)
                tiles[last]["norms"].append((h, si, f, gq[0], len(gq)))
                for qc in gq:
                    norm_due[(h, qc)] = last
        row_groups = oh_banks(G - 1, si)
        if len(row_groups) > 2:
            for gq in row_groups:
                due = max(norm_due[(h, qc)] for h in range(G) for qc in gq)
                tiles[due]["dmas"].append((si, gq[0], gq[-1] + 1))
        else:
            due = max(norm_due.values())
            nqc = ceil((segments[si][1] - segments[si][0]) / 128)
            tiles[due]["dmas"].append((si, 0, nqc))

    return tiles, nkt_tot, qc_bank


def _build_program(T, segments):
    f32 = mybir.dt.float32
    bf16 = mybir.dt.bfloat16

    tiles, nkt_tot, qc_bank = _plan(segments)
    L0 = segments[0][1] - segments[0][0]
    use_w0 = L0 >= 1024

    nc = bacc.Bacc(
        "TRN2",
        target_bir_lowering=False,
        debug=False,
        enable_asserts=False,
        num_devices=8,
    )
    qT_d = nc.dram_tensor("qT", [128, T], bf16, kind="ExternalInput").ap()
    kT_d = nc.dram_tensor("kT", [64, T], bf16, kind="ExternalInput").ap()
    vt_d = nc.dram_tensor("vt", [128, nkt_tot * 65], bf16, kind="ExternalInput").ap()
    if use_w0:
        w0_d = nc.dram_tensor("w0", [64, 128 + W0Q], bf16, kind="ExternalInput").ap()
    o_d = nc.dram_tensor("o", [T, G * D], f32, kind="ExternalOutput").ap()

    with tile.TileContext(nc) as tc, ExitStack() as ctx:
        const = ctx.enter_context(tc.tile_pool(name="const", bufs=1))
        inpool = ctx.enter_context(tc.tile_pool(name="inp", bufs=1))
        sepool = ctx.enter_context(tc.tile_pool(name="se", bufs=6))
        sdpool = ctx.enter_context(tc.tile_pool(name="sed", bufs=6))
        opool = ctx.enter_context(tc.tile_pool(name="osb", bufs=2))
        rpool = ctx.enter_context(tc.tile_pool(name="rcp", bufs=4))
        ps_a = ctx.enter_context(tc.tile_pool(name="ps_a", bufs=2, space="PSUM"))
        ps_d = ctx.enter_context(tc.tile_pool(name="ps_d", bufs=2, space="PSUM"))
        ps_o = ctx.enter_context(tc.tile_pool(name="ps_o", bufs=2, space="PSUM"))

        # trimask[p, c] = 1 if c >= p else 0 (valid = query col >= key partition)
        trimask = const.tile([128, 128], bf16)
        nc.gpsimd.memset(trimask, 0.0)
        nc.gpsimd.affine_select(
            out=trimask,
            in_=trimask,
            compare_op=mybir.AluOpType.is_gt,
            fill=1.0,
            base=0,
            pattern=[[-1, 128]],
            channel_multiplier=1,
        )

        qT = inpool.tile([128, T], bf16)
        kT = inpool.tile([128, T], bf16)
        vt = inpool.tile([128, nkt_tot * 65], bf16)
        vtv = vt.rearrange("p (n k) -> p n k", k=65)
        if use_w0:
            w0 = inpool.tile([64, 128 + W0Q], bf16, padded_shape=[128, 128 + W0Q])

        # ---- input DMAs: all issued up front ----
        # kT halves are loaded separately (h0 rows 0:64 first) so head 0's QK
        # stream unblocks as early as possible; w0 covers the whole first
        # (seg0, h0) non-diagonal stream so tiles 0..3 only depend on 1-2 DMAs
        s00, s01 = segments[0]
        kg = 0
        if use_w0:
            nc.sync.dma_start(w0, w0_d)
        first = True
        for (s0, s1) in segments:
            nkt = ceil((s1 - s0) / 128)
            nc.sync.dma_start(kT[0:64, s0:s1], kT_d[:, s0:s1])
            if first and use_w0:
                nc.sync.dma_start(
                    vt[:, kg * 65 : (kg + nkt) * 65],
                    vt_d[:, kg * 65 : (kg + nkt) * 65],
                )
                nc.sync.dma_start(qT[:, s0:s1], qT_d[:, s0:s1])
            else:
                nc.sync.dma_start(qT[:, s0:s1], qT_d[:, s0:s1])
                nc.sync.dma_start(
                    vt[:, kg * 65 : (kg + nkt) * 65],
                    vt_d[:, kg * 65 : (kg + nkt) * 65],
                )
            nc.sync.dma_start(kT[64:128, s0:s1], kT_d[:, s0:s1])
            kg += nkt
            first = False

        # ---- main stream ----
        se_tiles = {}
        oh_tiles = {}
        osb_tiles = {}

        def get_oh(h, si, f):
            key = (h, si, f)
            if key not in oh_tiles:
                oh_tiles[key] = ps_o.tile(
                    [128, 512], f32, tag="oh", name=f"oh_{h}_{si}_{f}"
                )
            return oh_tiles[key]

        def get_osb(si):
            if si not in osb_tiles:
                nqc = ceil((segments[si][1] - segments[si][0]) / 128)
                osb_tiles[si] = opool.tile(
                    [128, nqc * 128], f32, tag="osb", name=f"osb_{si}"
                )
            return osb_tiles[si]

        def emit_qk(g, sp):
            # diagonal subpieces first: the DVE exp2 pass only needs those,
            # so it can start while the rest of the tile's QKs still run
            dcols = set()
            for (c0, w) in tiles[g]["ts"]:
                dcols.update(range(c0, c0 + w, 128))
            tiles[g]["qk"].sort(key=lambda p: (p[6] not in dcols, p[6]))
            for (h, s0, klo, kp, qa, qb, col, from_w0) in tiles[g]["qk"]:
                lhsT = (
                    w0[0:64, 0:kp]
                    if (from_w0 and klo == 0)
                    else kT[64 * h : 64 * h + 64, s0 + klo : s0 + klo + kp]
                )
                rhs = (
                    w0[0:64, 128 + qa : 128 + qb]
                    if from_w0
                    else qT[64 * h : 64 * h + 64, s0 + qa : s0 + qb]
                )
                nc.tensor.matmul(
                    sp[:kp, col : col + qb - qa],
                    lhsT,
                    rhs,
                    start=True,
                    stop=True,
                    tile_position=(64 * h, 0),
                )

        def emit_post(g):
            t = tiles[g]
            se = se_tiles.pop(g)
            for (h, si, qc, col, cw, ktg, kp, start, stop) in t["pv"]:
                f, fbase = qc_bank[(h, si, qc)]
                oh = get_oh(h, si, f)
                lq = qc - fbase
                nc.tensor.matmul(
                    oh[:cw, 128 * lq : 128 * lq + 65],
                    se[:kp, col : col + cw],
                    vtv[:kp, ktg, :],
                    start=start,
                    stop=stop,
                )
            for (h, si, f, qc0, nq) in t["norms"]:
                oh = oh_tiles[(h, si, f)]
                osb = get_osb(si)
                lq = 0  # bank-local offset: each norm group is a whole oh tile
                uniq = f"{h}_{si}_{qc0}"
                rcp = rpool.tile([128, 8], f32, tag="rcp", name=f"rcp_{uniq}")
                nc.vector.reciprocal(
                    rcp[:, :nq],
                    oh[:, 128 * lq + 64 : 128 * (lq + nq - 1) + 65 : 128],
                )
                ohv = oh.rearrange("p (c k) -> p c k", k=128)[:, lq : lq + nq, 0:D]
                rv = rcp[:, :nq].rearrange("p (c k) -> p c k", k=1)
                rv, ohv = bass.broadcast_tensor_aps(rv, ohv)
                nc.vector.tensor_mul(
                    osb.rearrange("p (c k) -> p c k", k=128)[
                        :, qc0 : qc0 + nq, D * h : D * h + D
                    ],
                    ohv,
                    rv,
                )
            for (si, c0, c1) in t["dmas"]:
                s0 = segments[si][0]
                osb = osb_tiles[si]
                L = segments[si][1] - s0
                r0, r1 = 128 * c0, min(128 * c1, L)
                nfc = (r1 - r0) // 128
                if nfc:
                    nc.sync.dma_start(
                        o_d[s0 + r0 : s0 + r0 + nfc * 128, :].rearrange(
                            "(c p) k -> p c k", p=128
                        ),
                        osb.rearrange("p (c k) -> p c k", k=128)[:, c0 : c0 + nfc, :],
                    )
                if (r1 - r0) % 128:
                    rr = r0 + nfc * 128
                    nc.sync.dma_start(
                        o_d[s0 + rr : s0 + r1, :],
                        osb[: r1 - rr, 128 * (c0 + nfc) : 128 * (c0 + nfc) + G * D],
                    )

        # Schraudolph exp2: exp(SCALE*s) = 2^(SCALE*s*log2e), assembled as
        # an int16 bit pattern that IS the bf16 weight: i16 = trunc(t*2^7 +
        # ((127<<7) - C)). ~3% max weight error; used on diagonal blocks
        # (query chunk 0 stays exact on ScalarE) and a slice of non-diagonal
        # cols to balance the ScalarE and Vector engines.
        LOG2E = 1.4426950408889634
        A_TS = float(SCALE * LOG2E * 128.0)
        B_TS = float((127 << 7) - 2.8)
        i16dt = mybir.dt.int16

        for g in range(len(tiles)):
            t = tiles[g]
            sz = t["sz"]
            if t["kind"] == "a":
                sp = ps_a.tile([128, TILE_A], f32, tag="spa", name=f"spa_{g}")
                se = sepool.tile([128, TILE_A], bf16, tag="se", name=f"se_{g}")
            else:
                sp = ps_d.tile([128, TILE_D], f32, tag="spd", name=f"spd_{g}")
                se = sdpool.tile([128, TILE_D], bf16, tag="sed", name=f"sed_{g}")
            emit_qk(g, sp)
            se_tiles[g] = se
            # DVE exp2 bit trick straight into bf16: i16 = t*2^7 + bias is
            # the bit pattern of ~2^t in bf16 (one tensor_scalar, no copy)
            for (c0, w) in t["ts"]:
                nc.vector.tensor_scalar(
                    se[:, c0 : c0 + w].bitcast(i16dt),
                    sp[:, c0 : c0 + w],
                    A_TS,
                    B_TS,
                    mybir.AluOpType.mult,
                    mybir.AluOpType.add,
                )
            # causal mask for diagonal chunks (cheap bf16 multiply; Pool for
            # mid-stream tiles, DVE for the latency-critical closing tiles)
            for (c0, n, fm) in t["masks"]:
                sev = se[:, c0 : c0 + 128 * n].rearrange("p (c k) -> p c k", k=128)
                triv = trimask.rearrange("p (c k) -> p c k", k=128)
                triv, sev = bass.broadcast_tensor_aps(triv, sev)
                eng = nc.vector if fm else nc.gpsimd
                eng.tensor_mul(
                    se[:, c0 : c0 + 128 * n].rearrange("p (c k) -> p c k", k=128),
                    sev,
                    triv,
                )
            if g >= 2:
                emit_post(g - 2)
            # ScalarE exp over the complement of the DVE ranges
            pos = 0
            acts = []
            for (c0, w) in t["ts"]:
                if c0 > pos:
                    acts.append((pos, c0))
                pos = c0 + w
            if pos < sz:
                acts.append((pos, sz))
            for (a, b) in acts:
                nc.scalar.activation(
                    se[:, a:b],
                    sp[:, a:b],
                    mybir.ActivationFunctionType.Exp,
                    scale=SCALE,
                )
            # mask for ScalarE-path diagonal chunks (query chunk 0)
            for (c, fm) in t["pmask"]:
                eng = nc.vector if fm else nc.gpsimd
                eng.tensor_mul(se[:, c : c + 128], se[:, c : c + 128], trimask)
        emit_post(len(tiles) - 2)
        emit_post(len(tiles) - 1)

    nc.compile()
    return nc


def _segments_from_cu(cu_seqlens, T):
    edges = sorted(set([0, T] + [int(c) for c in cu_seqlens if 0 < int(c) < T]))
    return [(edges[i], edges[i + 1]) for i in range(len(edges) - 1)]


def kernel(q, k, v, k_cache, v_cache, slot_mapping, cu_seqlens):
    global LAST_RESULT
    T = q.shape[0]
    nslots = k_cache.shape[0]

    # Emulate scatter-then-gather through the paged cache: for duplicate slots
    # the last writer wins, so token i reads back k[lastw[slot[i]]].
    slot = np.asarray(slot_mapping, dtype=np.int64)
    lastw = np.zeros(nslots, dtype=np.int64)
    lastw[slot] = np.arange(T)
    lw = lastw[slot]
    k_eff = np.asarray(k)[lw]
    v_eff = np.asarray(v)[lw]

    segments = _segments_from_cu(np.asarray(cu_seqlens), T)
    key = (T, tuple(segments))
    if key not in _PROGRAM_CACHE:
        _PROGRAM_CACHE[key] = _build_program(T, segments)
    nc = _PROGRAM_CACHE[key]

    bf = ml_dtypes.bfloat16
    qh = np.ascontiguousarray(
        np.asarray(q).reshape(T, NKV * G, D).transpose(1, 2, 0)
    ).astype(bf)  # [16, 64, T]
    kh = np.ascontiguousarray(k_eff.reshape(T, NKV, D).transpose(1, 2, 0)).astype(bf)
    vh = v_eff.reshape(T, NKV, D).astype(bf)  # [T, 8, 64]

    # vt: per-segment 128-row tiling of v rows, with a ones column at k=64
    nkt_tot = sum(ceil((s1 - s0) / 128) for (s0, s1) in segments)
    L0 = segments[0][1] - segments[0][0]
    use_w0 = L0 >= 1024

    in_maps = []
    for h in range(NKV):
        qT = np.ascontiguousarray(qh[2 * h : 2 * h + 2].reshape(128, T))
        kT = np.ascontiguousarray(kh[h])  # [64, T]
        vt = np.zeros((128, nkt_tot, 65), dtype=bf)
        kg = 0
        for (s0, s1) in segments:
            L = s1 - s0
            for kt in range(ceil(L / 128)):
                klo = s0 + 128 * kt
                kp = min(128, s1 - klo)
                vt[:kp, kg, :D] = vh[klo : klo + kp, h, :]
                vt[:, kg, D] = 1.0
                kg += 1
        m = {
            "qT": qT,
            "kT": kT,
            "vt": np.ascontiguousarray(vt.reshape(128, nkt_tot * 65)),
        }
        if use_w0:
            s00 = segments[0][0]
            w0 = np.concatenate(
                [kT[:, s00 : s00 + 128], qT[0:64, s00 : s00 + W0Q]], axis=1
            )
            m["w0"] = np.ascontiguousarray(w0)
        in_maps.append(m)

    res = run_bass_kernel_spmd(nc, in_maps, core_ids=list(range(8)), trace=TRACE)
    LAST_RESULT = res

    out = np.empty((T, NKV * G * D), dtype=np.float32)
    ov = out.reshape(T, NKV, G * D)
    for h in range(NKV):
        ov[:, h, :] = res.results[h]["o"]
    return out


# revision 55
# speedup vs baseline: 1.4425x; 1.0274x over previous
"""Paged-KV varlen causal GQA attention for Trainium2, sharded over 8 NeuronCores.

Problem (hardcoded from spec): T=4096 tokens, 16 q heads / 8 kv heads, head_dim=64,
scale=0.125. k/v are scattered into paged caches via slot_mapping, read back, and
causal varlen attention (segments from cu_seqlens) is computed.

Sharding: tensor-parallel over kv heads -- core h gets kv head h and its 2 GQA
query heads. slot_mapping / cu_seqlens handled on host (index math only).

Device kernel (per core): a single flat stream of score tiles [128, <=1536] f32
(2 PSUM buffers x 3 banks) packed with (head, key-tile) pieces:
  sp[keys, queries] = kT.T @ qT          (PE, per piece, bank-split)
  se = exp(0.125 * sp)                   (ScalarE, one exp per score tile -- the
                                          bottleneck engine; tiles are as wide as
                                          PSUM allows to amortize instr overhead)
  diagonal blocks: se *= trimask         (VectorE, one batched mul per block)
  o[q, 0:65] += se[:,qc].T @ [v | 1]     (PE with se as STATIONARY: output lands
                                          query-major, col 64 = softmax denom --
                                          no transposes / PSUM->SBUF copies)
  osb[q, d] = o[q, d] * 1/o[q, 64]       (VectorE reciprocal + broadcast mul)
"""

import os
from contextlib import ExitStack
from math import ceil

import numpy as np
import ml_dtypes

import concourse.bass as bass
import concourse.mybir as mybir
import concourse.tile as tile
from concourse import bacc
from concourse.bass_utils import run_bass_kernel_spmd

NKV = 8
G = 2
D = 64
SCALE = 0.125

TILE_A = 1024  # ScalarE score tile width (2 PSUM banks of f32, 2 buffers)
TILE_D = 512   # DVE exp2 score tile width (1 PSUM bank, 2 buffers)
BANK = 512     # f32 cols per PSUM bank
W0Q = 1024     # warmup tensor covers kT[0:128] + qT[0:1024] of segment 0

# test.py pokes these for profiling
TRACE = bool(int(os.environ.get("KERNEL_TRACE", "0")))
LAST_RESULT = None

_PROGRAM_CACHE = {}


def _plan(segments):
    """Pack the per-(segment, head) score work into TWO independent tile
    streams, interleaved in one creation-order list:
      kind 'a': ScalarE-exp tiles, [128, <=1024] f32 (2 PSUM banks x 2 bufs)
      kind 'd': DVE-exp2 tiles,    [128, <=512]  f32 (1 PSUM bank  x 2 bufs)
    Each engine paces its own stream, so an sp slot is freed by exactly one
    reader and the pipelines decouple.

    Tile dicts: kind, sz, qk, ts, masks, pmask, pv, norms, dmas.
    """
    tiles = []
    cur = {"a": None, "d": None}
    acap = [384, 640]  # warmup caps for the first A tiles, then TILE_A
    chains = {}  # (h, si, qc) -> [ [tile_idx, col, cw, ktg, kp], ... ]

    def new_tile(kind):
        t = {
            "kind": kind, "idx": len(tiles), "sz": 0, "qk": [], "ts": [],
            "masks": [], "pmask": [], "pv": [], "norms": [], "dmas": [],
        }
        tiles.append(t)
        cur[kind] = t
        return t

    def close(kind):
        cur[kind] = None

    def cap(kind):
        if kind == "d":
            return TILE_D
        na = sum(1 for t in tiles if t["kind"] == "a")
        if cur["a"] is not None:
            na -= 1
        return acap[na] if na < len(acap) else TILE_A

    # global key-tile counter (vt layout is per-segment tiled on host)
    seg_ktg0 = []
    nkt_tot = 0
    for (s0, s1) in segments:
        seg_ktg0.append(nkt_tot)
        nkt_tot += ceil((s1 - s0) / 128)

    def place_span(kind, h, si, kt, qa, qb, w0_ok):
        """Place one query span of key tile kt into the `kind` stream,
        splitting at tile caps and PSUM bank edges; register PV chunks.
        Returns [(tile, col, ncols)] placements."""
        s0, s1 = segments[si]
        L = s1 - s0
        klo = 128 * kt
        kp = min(128, L - klo)
        ktg = seg_ktg0[si] + kt
        placements = []
        q = qa
        while q < qb:
            if cur[kind] is None or cur[kind]["sz"] >= cap(kind):
                new_tile(kind)
            t = cur[kind]
            room = cap(kind) - t["sz"]
            take = min(qb - q, room)
            col = t["sz"]
            placements.append((t, col, take))
            qq = q
            while qq < q + take:
                c = col + (qq - q)
                lim = qq + min(q + take - qq, BANK - (c % BANK))
                use_w0 = w0_ok and qq < W0Q
                if use_w0 and lim > W0Q:
                    lim = W0Q
                t["qk"].append((h, s0, klo, kp, qq, lim, c, use_w0))
                qq = lim
            for qs in range(q, q + take, 128):
                qc = qs // 128
                cw = min(128, qb - qs, q + take - qs)
                chains.setdefault((h, si, qc), []).append(
                    [t["idx"], col + (qs - q), cw, ktg, kp]
                )
            t["sz"] += take
            if t["sz"] % 128:
                t["sz"] += 128 - t["sz"] % 128
            q += take
        return placements

    def place_diag(h, si, kts, act, w0_ok=False, fast_mask=False):
        """Diagonal chunks: kt0 (act=True) goes on the A stream with a
        mask-multiply (pmask); others go on the D stream (DVE exp2) with
        the causal mask applied by Pool (or DVE when fast_mask)."""
        s0, s1 = segments[si]
        L = s1 - s0
        for kt in kts:
            klo = 128 * kt
            kp = min(128, L - klo)
            pl = place_span("a" if act else "d", h, si, kt, klo, klo + kp, w0_ok)
            for (t, col, n) in pl:
                if act:
                    t["pmask"].append((col, fast_mask))
                else:
                    t["ts"].append((col, n))
                    t["masks"].append((col, n // 128, fast_mask))

    def take_nd(kind, h, si, ndq, ncols, w0_ok=False):
        """Consume ncols of non-diagonal pieces into the given stream; D-
        stream cols get ts entries (DVE exp2, no mask needed)."""
        left = ncols
        while left > 0:
            kt, qa, qb = ndq[0]
            take = min(qb - qa, left)
            pl = place_span(kind, h, si, kt, qa, qa + take, w0_ok)
            if kind == "d":
                for (t, col, n) in pl:
                    t["ts"].append((col, n))
            if qa + take == qb:
                ndq.pop(0)
            else:
                ndq[0][1] = qa + take
            left -= take

    nseg = len(segments)
    for si, (s0, s1) in enumerate(segments):
        L = s1 - s0
        nkt = ceil(L / 128)
        nqc = ceil(L / 128)
        for h in range(G):
            first_sh = si == 0 and h == 0 and L >= 1024
            last_sh = (
                si == nseg - 1 and h == G - 1 and nqc >= 8 and L % 128 == 0
            )
            nd = [kt for kt in range(nkt - 1) if 128 * (kt + 1) < L]
            fast = L == 1024 and nkt == 8
            if fast and not last_sh:
                w0_ok = first_sh
                ndq = [[kt, 128 * (kt + 1), L] for kt in nd]
                # route the last 640 nd cols (late stripes of kt4..6, well
                # diluted query chunks) through the DVE stream; with the kt0
                # diagonal on the A stream both streams then hold an exact
                # number of tiles per (seg, head), which keeps oh-bank norms
                # strictly before the next tenant's first PV in program order
                extras = [[4, 768, 1024], [5, 768, 1024], [6, 896, 1024]]
                ndq[4][2] = 768
                ndq[5][2] = 768
                ndq = [p for p in ndq if p[1] < p[2] and p[0] != 6]
                take_nd("a", h, si, ndq, 896, w0_ok)
                place_diag(h, si, [0], True, w0_ok)
                place_diag(h, si, [1, 2, 3, 4], False, w0_ok)
                take_nd("a", h, si, ndq, 1024, w0_ok)
                place_diag(h, si, [5, 6, 7], False, w0_ok)
                take_nd("d", h, si, extras, 640, w0_ok)
                take_nd("a", h, si, ndq, 1024, w0_ok)
                close("a")
                close("d")
            elif fast and last_sh:
                # tail: qchunk 7's pieces and the final diagonal chunk come
                # last, in their own small tiles, for a short closing chain
                ndqa = [[kt, 128 * (kt + 1), L - 128] for kt in nd]
                ndqb = [[kt, L - 128, L] for kt in nd]
                extras = [[4, 640, 896], [5, 768, 896]]
                ndqa[4][2] = 640
                ndqa[5] = [5, 768, 768]
                ndqa = [p for p in ndqa if p[1] < p[2]]
                take_nd("a", h, si, ndqa, 1024)
                place_diag(h, si, [1, 2, 3], False, fast_mask=True)
                take_nd("a", h, si, ndqa, 1024)
                place_diag(h, si, [4, 5, 6], False, fast_mask=True)
                take_nd("d", h, si, extras, 384)
                take_nd("a", h, si, ndqa, 256)
                place_diag(h, si, [0], True, fast_mask=True)
                close("a")
                take_nd("a", h, si, ndqb, 896)
                close("a")
                close("d")
                place_diag(h, si, [7], False, fast_mask=True)
                close("d")
            else:
                # generic fallback: everything on the A stream, diag last,
                # masks on Pool
                for kt in nd:
                    place_span("a", h, si, kt, 128 * (kt + 1), L, first_sh)
                place_diag(h, si, [0], True, first_sh)
                place_diag(h, si, list(range(1, nkt)), False, first_sh)

    # coalesce adjacent ts / mask ranges per tile
    for t in tiles:
        for key in ("ts",):
            t[key].sort()
            merged = []
            for (c0, w) in t[key]:
                if merged and merged[-1][0] + merged[-1][1] == c0:
                    merged[-1][1] += w
                else:
                    merged.append([c0, w])
            t[key] = [tuple(m) for m in merged]
        t["masks"].sort()
        merged = []
        for (c0, n, fm) in t["masks"]:
            if merged and merged[-1][0] + 128 * merged[-1][1] == c0 and merged[-1][2] == fm:
                merged[-1][1] += n
            else:
                merged.append([c0, n, fm])
        t["masks"] = [tuple(m) for m in merged]

    # qchunk -> oh-bank mapping: 4 chunks per PSUM bank, except the last
    # (segment, head) where the final chunk gets its own bank so its
    # accumulation group closes independently
    def oh_banks(h, si):
        L = segments[si][1] - segments[si][0]
        nqc = ceil(L / 128)
        if si == nseg - 1 and h == G - 1 and nqc == 8:
            return [[0, 1, 2], [3, 4, 5, 6], [7]]
        return [
            list(range(4 * f, min(4 * f + 4, nqc))) for f in range(ceil(nqc / 4))
        ]

    qc_bank = {}  # (h, si, qc) -> (f, fbase)
    for si in range(nseg):
        for h in range(G):
            for f, qcs in enumerate(oh_banks(h, si)):
                for qc in qcs:
                    qc_bank[(h, si, qc)] = (f, qcs[0])

    # start/stop flags are PER PSUM BANK (zero region), not per qchunk chain:
    # start_tensor_calc pends-zero the whole 2KB bank, so only the
    # chronologically first matmul into an oh tile may set it, and only the
    # last sets stop. Per-byte lazy zeroing makes each chain's first write a
    # plain store and later writes accumulates, regardless of interleaving.
    oh_groups = {}
    for (h, si, qc), ch in chains.items():
        for e in ch:
            oh_groups.setdefault((h, si, qc_bank[(h, si, qc)][0]), []).append(e)
    for grp in oh_groups.values():
        grp.sort(key=lambda e: (e[0], e[1]))
        for e in grp:
            e.append(e is grp[0])
            e.append(e is grp[-1])
    for (h, si, qc), ch in chains.items():
        for e in ch:
            tiles[e[0]]["pv"].append((h, si, qc, e[1], e[2], e[3], e[4], e[5], e[6]))

    # order pv entries in each tile by emission col to keep per-chain order
    for t in tiles:
        t["pv"].sort(key=lambda p: (p[3], p[0], p[1], p[2]))

    # norm groups (one per oh bank): due after max last tile over the chains
    for si in range(nseg):
        norm_due = {}
        for h in range(G):
            for f, gq in enumerate(oh_banks(h, si)):
                last = max(chains[(h, si, qc)][-1][0] for qc in gq)
                tiles[last]["norms"].append((h, si, f, gq[0], len(gq)))
                for qc in gq:
                    norm_due[(h, qc)] = last
        row_groups = oh_banks(G - 1, si)
        if len(row_groups) > 2:
            for gq in row_groups:
                due = max(norm_due[(h, qc)] for h in range(G) for qc in gq)
                tiles[due]["dmas"].append((si, gq[0], gq[-1] + 1))
        else:
            due = max(norm_due.values())
            nqc = ceil((segments[si][1] - segments[si][0]) / 128)
            tiles[due]["dmas"].append((si, 0, nqc))

    return tiles, nkt_tot, qc_bank


def _build_program(T, segments):
    f32 = mybir.dt.float32
    bf16 = mybir.dt.bfloat16

    tiles, nkt_tot, qc_bank = _plan(segments)
    L0 = segments[0][1] - segments[0][0]
    use_w0 = L0 >= 1024

    nc = bacc.Bacc(
        "TRN2",
        target_bir_lowering=False,
        debug=False,
        enable_asserts=False,
        num_devices=8,
    )
    qT_d = nc.dram_tensor("qT", [128, T], bf16, kind="ExternalInput").ap()
    kT_d = nc.dram_tensor("kT", [64, T], bf16, kind="ExternalInput").ap()
    vt_d = nc.dram_tensor("vt", [128, nkt_tot * 65], bf16, kind="ExternalInput").ap()
    if use_w0:
        w0_d = nc.dram_tensor("w0", [64, 128 + W0Q], bf16, kind="ExternalInput").ap()
    o_d = nc.dram_tensor("o", [T, G * D], f32, kind="ExternalOutput").ap()

    with tile.TileContext(nc) as tc, ExitStack() as ctx:
        const = ctx.enter_context(tc.tile_pool(name="const", bufs=1))
        inpool = ctx.enter_context(tc.tile_pool(name="inp", bufs=1))
        sepool = ctx.enter_context(tc.tile_pool(name="se", bufs=6))
        sdpool = ctx.enter_context(tc.tile_pool(name="sed", bufs=6))
        opool = ctx.enter_context(tc.tile_pool(name="osb", bufs=2))
        rpool = ctx.enter_context(tc.tile_pool(name="rcp", bufs=4))
        ps_a = ctx.enter_context(tc.tile_pool(name="ps_a", bufs=2, space="PSUM"))
        ps_d = ctx.enter_context(tc.tile_pool(name="ps_d", bufs=2, space="PSUM"))
        ps_o = ctx.enter_context(tc.tile_pool(name="ps_o", bufs=2, space="PSUM"))

        # trimask[p, c] = 1 if c >= p else 0 (valid = query col >= key partition)
        trimask = const.tile([128, 128], bf16)
        nc.gpsimd.memset(trimask, 0.0)
        nc.gpsimd.affine_select(
            out=trimask,
            in_=trimask,
            compare_op=mybir.AluOpType.is_gt,
            fill=1.0,
            base=0,
            pattern=[[-1, 128]],
            channel_multiplier=1,
        )

        qT = inpool.tile([128, T], bf16)
        kT = inpool.tile([128, T], bf16)
        vt = inpool.tile([128, nkt_tot * 65], bf16)
        vtv = vt.rearrange("p (n k) -> p n k", k=65)
        if use_w0:
            w0 = inpool.tile([64, 128 + W0Q], bf16, padded_shape=[128, 128 + W0Q])

        # ---- input DMAs: all issued up front ----
        # kT halves are loaded separately (h0 rows 0:64 first) so head 0's QK
        # stream unblocks as early as possible; w0 covers the whole first
        # (seg0, h0) non-diagonal stream so tiles 0..3 only depend on 1-2 DMAs
        s00, s01 = segments[0]
        kg = 0
        if use_w0:
            nc.sync.dma_start(w0, w0_d)
        first = True
        for (s0, s1) in segments:
            nkt = ceil((s1 - s0) / 128)
            nc.sync.dma_start(kT[0:64, s0:s1], kT_d[:, s0:s1])
            if first and use_w0:
                nc.sync.dma_start(
                    vt[:, kg * 65 : (kg + nkt) * 65],
                    vt_d[:, kg * 65 : (kg + nkt) * 65],
                )
                nc.sync.dma_start(qT[:, s0:s1], qT_d[:, s0:s1])
            else:
                nc.sync.dma_start(qT[:, s0:s1], qT_d[:, s0:s1])
                nc.sync.dma_start(
                    vt[:, kg * 65 : (kg + nkt) * 65],
                    vt_d[:, kg * 65 : (kg + nkt) * 65],
                )
            nc.sync.dma_start(kT[64:128, s0:s1], kT_d[:, s0:s1])
            kg += nkt
            first = False

        # ---- main stream ----
        se_tiles = {}
        oh_tiles = {}
        osb_tiles = {}

        def get_oh(h, si, f):
            key = (h, si, f)
            if key not in oh_tiles:
                oh_tiles[key] = ps_o.tile(
                    [128, 512], f32, tag="oh", name=f"oh_{h}_{si}_{f}"
                )
            return oh_tiles[key]

        def get_osb(si):
            if si not in osb_tiles:
                nqc = ceil((segments[si][1] - segments[si][0]) / 128)
                osb_tiles[si] = opool.tile(
                    [128, nqc * 128], f32, tag="osb", name=f"osb_{si}"
                )
            return osb_tiles[si]

        def emit_qk(g, sp):
            # diagonal subpieces first: the DVE exp2 pass only needs those,
            # so it can start while the rest of the tile's QKs still run
            dcols = set()
            for (c0, w) in tiles[g]["ts"]:
                dcols.update(range(c0, c0 + w, 128))
            tiles[g]["qk"].sort(key=lambda p: (p[6] not in dcols, p[6]))
            for (h, s0, klo, kp, qa, qb, col, from_w0) in tiles[g]["qk"]:
                lhsT = (
                    w0[0:64, 0:kp]
                    if (from_w0 and klo == 0)
                    else kT[64 * h : 64 * h + 64, s0 + klo : s0 + klo + kp]
                )
                rhs = (
                    w0[0:64, 128 + qa : 128 + qb]
                    if from_w0
                    else qT[64 * h : 64 * h + 64, s0 + qa : s0 + qb]
                )
                nc.tensor.matmul(
                    sp[:kp, col : col + qb - qa],
                    lhsT,
                    rhs,
                    start=True,
                    stop=True,
                    tile_position=(64 * h, 0),
                )

        def emit_post(g):
            t = tiles[g]
            se = se_tiles.pop(g)
            for (h, si, qc, col, cw, ktg, kp, start, stop) in t["pv"]:
                f, fbase = qc_bank[(h, si, qc)]
                oh = get_oh(h, si, f)
                lq = qc - fbase
                nc.tensor.matmul(
                    oh[:cw, 128 * lq : 128 * lq + 65],
                    se[:kp, col : col + cw],
                    vtv[:kp, ktg, :],
                    start=start,
                    stop=stop,
                )
            for (h, si, f, qc0, nq) in t["norms"]:
                oh = oh_tiles[(h, si, f)]
                osb = get_osb(si)
                lq = 0  # bank-local offset: each norm group is a whole oh tile
                uniq = f"{h}_{si}_{qc0}"
                rcp = rpool.tile([128, 8], f32, tag="rcp", name=f"rcp_{uniq}")
                nc.vector.reciprocal(
                    rcp[:, :nq],
                    oh[:, 128 * lq + 64 : 128 * (lq + nq - 1) + 65 : 128],
                )
                ohv = oh.rearrange("p (c k) -> p c k", k=128)[:, lq : lq + nq, 0:D]
                rv = rcp[:, :nq].rearrange("p (c k) -> p c k", k=1)
                rv, ohv = bass.broadcast_tensor_aps(rv, ohv)
                nc.vector.tensor_mul(
                    osb.rearrange("p (c k) -> p c k", k=128)[
                        :, qc0 : qc0 + nq, D * h : D * h + D
                    ],
                    ohv,
                    rv,
                )
            for (si, c0, c1) in t["dmas"]:
                s0 = segments[si][0]
                osb = osb_tiles[si]
                L = segments[si][1] - s0
                r0, r1 = 128 * c0, min(128 * c1, L)
                nfc = (r1 - r0) // 128
                if nfc:
                    nc.sync.dma_start(
                        o_d[s0 + r0 : s0 + r0 + nfc * 128, :].rearrange(
                            "(c p) k -> p c k", p=128
                        ),
                        osb.rearrange("p (c k) -> p c k", k=128)[:, c0 : c0 + nfc, :],
                    )
                if (r1 - r0) % 128:
                    rr = r0 + nfc * 128
                    nc.sync.dma_start(
                        o_d[s0 + rr : s0 + r1, :],
                        osb[: r1 - rr, 128 * (c0 + nfc) : 128 * (c0 + nfc) + G * D],
                    )

        # Schraudolph exp2: exp(SCALE*s) = 2^(SCALE*s*log2e), assembled as
        # an int16 bit pattern that IS the bf16 weight: i16 = trunc(t*2^7 +
        # ((127<<7) - C)). ~3% max weight error; used on diagonal blocks
        # (query chunk 0 stays exact on ScalarE) and a slice of non-diagonal
        # cols to balance the ScalarE and Vector engines.
        LOG2E = 1.4426950408889634
        A_TS = float(SCALE * LOG2E * 128.0)
        B_TS = float((127 << 7) - 2.8)
        i16dt = mybir.dt.int16

        for g in range(len(tiles)):
            t = tiles[g]
            sz = t["sz"]
            if t["kind"] == "a":
                sp = ps_a.tile([128, TILE_A], f32, tag="spa", name=f"spa_{g}")
                se = sepool.tile([128, TILE_A], bf16, tag="se", name=f"se_{g}")
            else:
                sp = ps_d.tile([128, TILE_D], f32, tag="spd", name=f"spd_{g}")
                se = sdpool.tile([128, TILE_D], bf16, tag="sed", name=f"sed_{g}")
            emit_qk(g, sp)
            se_tiles[g] = se
            # DVE exp2 bit trick straight into bf16: i16 = t*2^7 + bias is
            # the bit pattern of ~2^t in bf16 (one tensor_scalar, no copy)
            for (c0, w) in t["ts"]:
                nc.vector.tensor_scalar(
                    se[:, c0 : c0 + w].bitcast(i16dt),
                    sp[:, c0 : c0 + w],
                    A_TS,
                    B_TS,
                    mybir.AluOpType.mult,
                    mybir.AluOpType.add,
                )
            # causal mask for diagonal chunks (cheap bf16 multiply; Pool for
            # mid-stream tiles, DVE for the latency-critical closing tiles)
            for (c0, n, fm) in t["masks"]:
                sev = se[:, c0 : c0 + 128 * n].rearrange("p (c k) -> p c k", k=128)
                triv = trimask.rearrange("p (c k) -> p c k", k=128)
                triv, sev = bass.broadcast_tensor_aps(triv, sev)
                eng = nc.vector if fm else nc.gpsimd
                eng.tensor_mul(
                    se[:, c0 : c0 + 128 * n].rearrange("p (c k) -> p c k", k=128),
                    sev,
                    triv,
                )
            if g >= 3:
                emit_post(g - 3)
            # ScalarE exp over the complement of the DVE ranges
            pos = 0
            acts = []
            for (c0, w) in t["ts"]:
                if c0 > pos:
                    acts.append((pos, c0))
                pos = c0 + w
            if pos < sz:
                acts.append((pos, sz))
            for (a, b) in acts:
                nc.scalar.activation(
                    se[:, a:b],
                    sp[:, a:b],
                    mybir.ActivationFunctionType.Exp,
                    scale=SCALE,
                )
            # mask for ScalarE-path diagonal chunks (query chunk 0)
            for (c, fm) in t["pmask"]:
                eng = nc.vector if fm else nc.gpsimd
                eng.tensor_mul(se[:, c : c + 128], se[:, c : c + 128], trimask)
        emit_post(len(tiles) - 3)
        emit_post(len(tiles) - 2)
        emit_post(len(tiles) - 1)

    nc.compile()
    return nc


def _segments_from_cu(cu_seqlens, T):
    edges = sorted(set([0, T] + [int(c) for c in cu_seqlens if 0 < int(c) < T]))
    return [(edges[i], edges[i + 1]) for i in range(len(edges) - 1)]


def kernel(q, k, v, k_cache, v_cache, slot_mapping, cu_seqlens):
    global LAST_RESULT
    T = q.shape[0]
    nslots = k_cache.shape[0]

    # Emulate scatter-then-gather through the paged cache: for duplicate slots
    # the last writer wins, so token i reads back k[lastw[slot[i]]].
    slot = np.asarray(slot_mapping, dtype=np.int64)
    lastw = np.zeros(nslots, dtype=np.int64)
    lastw[slot] = np.arange(T)
    lw = lastw[slot]
    k_eff = np.asarray(k)[lw]
    v_eff = np.asarray(v)[lw]

    segments = _segments_from_cu(np.asarray(cu_seqlens), T)
    key = (T, tuple(segments))
    if key not in _PROGRAM_CACHE:
        _PROGRAM_CACHE[key] = _build_program(T, segments)
    nc = _PROGRAM_CACHE[key]

    bf = ml_dtypes.bfloat16
    qh = np.ascontiguousarray(
        np.asarray(q).reshape(T, NKV * G, D).transpose(1, 2, 0)
    ).astype(bf)  # [16, 64, T]
    kh = np.ascontiguousarray(k_eff.reshape(T, NKV, D).transpose(1, 2, 0)).astype(bf)
    vh = v_eff.reshape(T, NKV, D).astype(bf)  # [T, 8, 64]

    # vt: per-segment 128-row tiling of v rows, with a ones column at k=64
    nkt_tot = sum(ceil((s1 - s0) / 128) for (s0, s1) in segments)
    L0 = segments[0][1] - segments[0][0]
    use_w0 = L0 >= 1024

    in_maps = []
    for h in range(NKV):
        qT = np.ascontiguousarray(qh[2 * h : 2 * h + 2].reshape(128, T))
        kT = np.ascontiguousarray(kh[h])  # [64, T]
        vt = np.zeros((128, nkt_tot, 65), dtype=bf)
        kg = 0
        for (s0, s1) in segments:
            L = s1 - s0
            for kt in range(ceil(L / 128)):
                klo = s0 + 128 * kt
                kp = min(128, s1 - klo)
                vt[:kp, kg, :D] = vh[klo : klo + kp, h, :]
                vt[:, kg, D] = 1.0
                kg += 1
        m = {
            "qT": qT,
            "kT": kT,
            "vt": np.ascontiguousarray(vt.reshape(128, nkt_tot * 65)),
        }
        if use_w0:
            s00 = segments[0][0]
            w0 = np.concatenate(
                [kT[:, s00 : s00 + 128], qT[0:64, s00 : s00 + W0Q]], axis=1
            )
            m["w0"] = np.ascontiguousarray(w0)
        in_maps.append(m)

    res = run_bass_kernel_spmd(nc, in_maps, core_ids=list(range(8)), trace=TRACE)
    LAST_RESULT = res

    out = np.empty((T, NKV * G * D), dtype=np.float32)
    ov = out.reshape(T, NKV, G * D)
    for h in range(NKV):
        ov[:, h, :] = res.results[h]["o"]
    return out


# revision 57
# speedup vs baseline: 1.4502x; 1.0053x over previous
"""Paged-KV varlen causal GQA attention for Trainium2, sharded over 8 NeuronCores.

Problem (hardcoded from spec): T=4096 tokens, 16 q heads / 8 kv heads, head_dim=64,
scale=0.125. k/v are scattered into paged caches via slot_mapping, read back, and
causal varlen attention (segments from cu_seqlens) is computed.

Sharding: tensor-parallel over kv heads -- core h gets kv head h and its 2 GQA
query heads. slot_mapping / cu_seqlens handled on host (index math only).

Device kernel (per core): a single flat stream of score tiles [128, <=1536] f32
(2 PSUM buffers x 3 banks) packed with (head, key-tile) pieces:
  sp[keys, queries] = kT.T @ qT          (PE, per piece, bank-split)
  se = exp(0.125 * sp)                   (ScalarE, one exp per score tile -- the
                                          bottleneck engine; tiles are as wide as
                                          PSUM allows to amortize instr overhead)
  diagonal blocks: se *= trimask         (VectorE, one batched mul per block)
  o[q, 0:65] += se[:,qc].T @ [v | 1]     (PE with se as STATIONARY: output lands
                                          query-major, col 64 = softmax denom --
                                          no transposes / PSUM->SBUF copies)
  osb[q, d] = o[q, d] * 1/o[q, 64]       (VectorE reciprocal + broadcast mul)
"""

import os
from contextlib import ExitStack
from math import ceil

import numpy as np
import ml_dtypes

import concourse.bass as bass
import concourse.mybir as mybir
import concourse.tile as tile
from concourse import bacc
from concourse.bass_utils import run_bass_kernel_spmd

NKV = 8
G = 2
D = 64
SCALE = 0.125

TILE_A = 1024  # ScalarE score tile width (2 PSUM banks of f32, 2 buffers)
TILE_D = 512   # DVE exp2 score tile width (1 PSUM bank, 2 buffers)
BANK = 512     # f32 cols per PSUM bank
W0Q = 1024     # warmup tensor covers kT[0:128] + qT[0:1024] of segment 0

# test.py pokes these for profiling
TRACE = bool(int(os.environ.get("KERNEL_TRACE", "0")))
LAST_RESULT = None

_PROGRAM_CACHE = {}


def _plan(segments):
    """Pack the per-(segment, head) score work into TWO independent tile
    streams, interleaved in one creation-order list:
      kind 'a': ScalarE-exp tiles, [128, <=1024] f32 (2 PSUM banks x 2 bufs)
      kind 'd': DVE-exp2 tiles,    [128, <=512]  f32 (1 PSUM bank  x 2 bufs)
    Each engine paces its own stream, so an sp slot is freed by exactly one
    reader and the pipelines decouple.

    Tile dicts: kind, sz, qk, ts, masks, pmask, pv, norms, dmas.
    """
    tiles = []
    cur = {"a": None, "d": None}
    acap = [384, 640]  # warmup caps for the first A tiles, then TILE_A
    chains = {}  # (h, si, qc) -> [ [tile_idx, col, cw, ktg, kp], ... ]

    def new_tile(kind):
        t = {
            "kind": kind, "idx": len(tiles), "sz": 0, "qk": [], "ts": [],
            "masks": [], "pmask": [], "pv": [], "norms": [], "dmas": [],
        }
        tiles.append(t)
        cur[kind] = t
        return t

    def close(kind):
        cur[kind] = None

    def cap(kind):
        if kind == "d":
            return TILE_D
        na = sum(1 for t in tiles if t["kind"] == "a")
        if cur["a"] is not None:
            na -= 1
        return acap[na] if na < len(acap) else TILE_A

    # global key-tile counter (vt layout is per-segment tiled on host)
    seg_ktg0 = []
    nkt_tot = 0
    for (s0, s1) in segments:
        seg_ktg0.append(nkt_tot)
        nkt_tot += ceil((s1 - s0) / 128)

    def place_span(kind, h, si, kt, qa, qb, w0_ok):
        """Place one query span of key tile kt into the `kind` stream,
        splitting at tile caps and PSUM bank edges; register PV chunks.
        Returns [(tile, col, ncols)] placements."""
        s0, s1 = segments[si]
        L = s1 - s0
        klo = 128 * kt
        kp = min(128, L - klo)
        ktg = seg_ktg0[si] + kt
        placements = []
        q = qa
        while q < qb:
            if cur[kind] is None or cur[kind]["sz"] >= cap(kind):
                new_tile(kind)
            t = cur[kind]
            room = cap(kind) - t["sz"]
            take = min(qb - q, room)
            col = t["sz"]
            placements.append((t, col, take))
            qq = q
            while qq < q + take:
                c = col + (qq - q)
                lim = qq + min(q + take - qq, BANK - (c % BANK))
                use_w0 = w0_ok and qq < W0Q
                if use_w0 and lim > W0Q:
                    lim = W0Q
                t["qk"].append((h, s0, klo, kp, qq, lim, c, use_w0))
                qq = lim
            for qs in range(q, q + take, 128):
                qc = qs // 128
                cw = min(128, qb - qs, q + take - qs)
                chains.setdefault((h, si, qc), []).append(
                    [t["idx"], col + (qs - q), cw, ktg, kp]
                )
            t["sz"] += take
            if t["sz"] % 128:
                t["sz"] += 128 - t["sz"] % 128
            q += take
        return placements

    def place_diag(h, si, kts, act, w0_ok=False, fast_mask=False):
        """Diagonal chunks: kt0 (act=True) goes on the A stream with a
        mask-multiply (pmask); others go on the D stream (DVE exp2) with
        the causal mask applied by Pool (or DVE when fast_mask)."""
        s0, s1 = segments[si]
        L = s1 - s0
        for kt in kts:
            klo = 128 * kt
            kp = min(128, L - klo)
            pl = place_span("a" if act else "d", h, si, kt, klo, klo + kp, w0_ok)
            for (t, col, n) in pl:
                if act:
                    t["pmask"].append((col, fast_mask))
                else:
                    t["ts"].append((col, n))
                    t["masks"].append((col, n // 128, fast_mask))

    def take_nd(kind, h, si, ndq, ncols, w0_ok=False):
        """Consume ncols of non-diagonal pieces into the given stream; D-
        stream cols get ts entries (DVE exp2, no mask needed)."""
        left = ncols
        while left > 0:
            kt, qa, qb = ndq[0]
            take = min(qb - qa, left)
            pl = place_span(kind, h, si, kt, qa, qa + take, w0_ok)
            if kind == "d":
                for (t, col, n) in pl:
                    t["ts"].append((col, n))
            if qa + take == qb:
                ndq.pop(0)
            else:
                ndq[0][1] = qa + take
            left -= take

    nseg = len(segments)
    for si, (s0, s1) in enumerate(segments):
        L = s1 - s0
        nkt = ceil(L / 128)
        nqc = ceil(L / 128)
        for h in range(G):
            first_sh = si == 0 and h == 0 and L >= 1024
            last_sh = (
                si == nseg - 1 and h == G - 1 and nqc >= 8 and L % 128 == 0
            )
            nd = [kt for kt in range(nkt - 1) if 128 * (kt + 1) < L]
            fast = L == 1024 and nkt == 8
            if fast and not last_sh:
                w0_ok = first_sh
                ndq = [[kt, 128 * (kt + 1), L] for kt in nd]
                # route the last 640 nd cols (late stripes of kt4..6, well
                # diluted query chunks) through the DVE stream; with the kt0
                # diagonal on the A stream both streams then hold an exact
                # number of tiles per (seg, head), which keeps oh-bank norms
                # strictly before the next tenant's first PV in program order
                extras = [[4, 768, 1024], [5, 768, 1024], [6, 896, 1024]]
                ndq[4][2] = 768
                ndq[5][2] = 768
                ndq = [p for p in ndq if p[1] < p[2] and p[0] != 6]
                take_nd("a", h, si, ndq, 896, w0_ok)
                place_diag(h, si, [0], True, w0_ok)
                place_diag(h, si, [1, 2, 3, 4], False, w0_ok)
                take_nd("a", h, si, ndq, 1024, w0_ok)
                place_diag(h, si, [5, 6, 7], False, w0_ok)
                take_nd("d", h, si, extras, 640, w0_ok)
                take_nd("a", h, si, ndq, 1024, w0_ok)
                close("a")
                close("d")
            elif fast and last_sh:
                # tail: qchunk 7's pieces and the final diagonal chunk come
                # last, in their own small tiles, for a short closing chain
                ndqa = [[kt, 128 * (kt + 1), L - 128] for kt in nd]
                ndqb = [[kt, L - 128, L] for kt in nd]
                extras = [[4, 640, 896], [5, 768, 896]]
                ndqa[4][2] = 640
                ndqa[5] = [5, 768, 768]
                ndqa = [p for p in ndqa if p[1] < p[2]]
                take_nd("a", h, si, ndqa, 1024)
                place_diag(h, si, [1, 2, 3], False, fast_mask=True)
                take_nd("a", h, si, ndqa, 1024)
                place_diag(h, si, [4, 5, 6], False, fast_mask=True)
                take_nd("d", h, si, extras, 384)
                take_nd("a", h, si, ndqa, 256)
                place_diag(h, si, [0], True, fast_mask=True)
                close("a")
                take_nd("a", h, si, ndqb, 896)
                close("a")
                close("d")
                place_diag(h, si, [7], False, fast_mask=True)
                close("d")
            else:
                # generic fallback: everything on the A stream, diag last,
                # masks on Pool
                for kt in nd:
                    place_span("a", h, si, kt, 128 * (kt + 1), L, first_sh)
                place_diag(h, si, [0], True, first_sh)
                place_diag(h, si, list(range(1, nkt)), False, first_sh)

    # coalesce adjacent ts / mask ranges per tile
    for t in tiles:
        for key in ("ts",):
            t[key].sort()
            merged = []
            for (c0, w) in t[key]:
                if merged and merged[-1][0] + merged[-1][1] == c0:
                    merged[-1][1] += w
                else:
                    merged.append([c0, w])
            t[key] = [tuple(m) for m in merged]
        t["masks"].sort()
        merged = []
        for (c0, n, fm) in t["masks"]:
            if merged and merged[-1][0] + 128 * merged[-1][1] == c0 and merged[-1][2] == fm:
                merged[-1][1] += n
            else:
                merged.append([c0, n, fm])
        t["masks"] = [tuple(m) for m in merged]

    # qchunk -> oh-bank mapping: 4 chunks per PSUM bank, except the last
    # (segment, head) where the final chunk gets its own bank so its
    # accumulation group closes independently
    def oh_banks(h, si):
        L = segments[si][1] - segments[si][0]
        nqc = ceil(L / 128)
        if si == nseg - 1 and h == G - 1 and nqc == 8:
            return [[0, 1, 2], [3, 4, 5, 6], [7]]
        return [
            list(range(4 * f, min(4 * f + 4, nqc))) for f in range(ceil(nqc / 4))
        ]

    qc_bank = {}  # (h, si, qc) -> (f, fbase)
    for si in range(nseg):
        for h in range(G):
            for f, qcs in enumerate(oh_banks(h, si)):
                for qc in qcs:
                    qc_bank[(h, si, qc)] = (f, qcs[0])

    # start/stop flags are PER PSUM BANK (zero region), not per qchunk chain:
    # start_tensor_calc pends-zero the whole 2KB bank, so only the
    # chronologically first matmul into an oh tile may set it, and only the
    # last sets stop. Per-byte lazy zeroing makes each chain's first write a
    # plain store and later writes accumulates, regardless of interleaving.
    oh_groups = {}
    for (h, si, qc), ch in chains.items():
        for e in ch:
            oh_groups.setdefault((h, si, qc_bank[(h, si, qc)][0]), []).append(e)
    for grp in oh_groups.values():
        grp.sort(key=lambda e: (e[0], e[1]))
        for e in grp:
            e.append(e is grp[0])
            e.append(e is grp[-1])
    for (h, si, qc), ch in chains.items():
        for e in ch:
            tiles[e[0]]["pv"].append((h, si, qc, e[1], e[2], e[3], e[4], e[5], e[6]))

    # order pv entries in each tile by emission col to keep per-chain order
    for t in tiles:
        t["pv"].sort(key=lambda p: (p[3], p[0], p[1], p[2]))

    # norm groups (one per oh bank): due after max last tile over the chains
    for si in range(nseg):
        norm_due = {}
        for h in range(G):
            for f, gq in enumerate(oh_banks(h, si)):
                last = max(chains[(h, si, qc)][-1][0] for qc in gq)
                tiles[last]["norms"].append((h, si, f, gq[0], len(gq)))
                for qc in gq:
                    norm_due[(h, qc)] = last
        row_groups = oh_banks(G - 1, si)
        if len(row_groups) > 2:
            for gq in row_groups:
                due = max(norm_due[(h, qc)] for h in range(G) for qc in gq)
                tiles[due]["dmas"].append((si, gq[0], gq[-1] + 1))
        else:
            due = max(norm_due.values())
            nqc = ceil((segments[si][1] - segments[si][0]) / 128)
            tiles[due]["dmas"].append((si, 0, nqc))

    return tiles, nkt_tot, qc_bank


def _build_program(T, segments):
    f32 = mybir.dt.float32
    bf16 = mybir.dt.bfloat16

    tiles, nkt_tot, qc_bank = _plan(segments)
    L0 = segments[0][1] - segments[0][0]
    use_w0 = L0 >= 1024

    nc = bacc.Bacc(
        "TRN2",
        target_bir_lowering=False,
        debug=False,
        enable_asserts=False,
        num_devices=8,
    )
    qT_d = nc.dram_tensor("qT", [128, T], bf16, kind="ExternalInput").ap()
    kT_d = nc.dram_tensor("kT", [64, T], bf16, kind="ExternalInput").ap()
    vt_d = nc.dram_tensor("vt", [128, nkt_tot * 65], bf16, kind="ExternalInput").ap()
    if use_w0:
        w0_d = nc.dram_tensor("w0", [64, 128 + W0Q], bf16, kind="ExternalInput").ap()
    o_d = nc.dram_tensor("o", [T, G * D], f32, kind="ExternalOutput").ap()

    with tile.TileContext(nc) as tc, ExitStack() as ctx:
        const = ctx.enter_context(tc.tile_pool(name="const", bufs=1))
        inpool = ctx.enter_context(tc.tile_pool(name="inp", bufs=1))
        sepool = ctx.enter_context(tc.tile_pool(name="se", bufs=8))
        sdpool = ctx.enter_context(tc.tile_pool(name="sed", bufs=8))
        opool = ctx.enter_context(tc.tile_pool(name="osb", bufs=3))
        rpool = ctx.enter_context(tc.tile_pool(name="rcp", bufs=6))
        ps_a = ctx.enter_context(tc.tile_pool(name="ps_a", bufs=2, space="PSUM"))
        ps_d = ctx.enter_context(tc.tile_pool(name="ps_d", bufs=2, space="PSUM"))
        ps_o = ctx.enter_context(tc.tile_pool(name="ps_o", bufs=2, space="PSUM"))

        # trimask[p, c] = 1 if c >= p else 0 (valid = query col >= key partition)
        trimask = const.tile([128, 128], bf16)
        nc.gpsimd.memset(trimask, 0.0)
        nc.gpsimd.affine_select(
            out=trimask,
            in_=trimask,
            compare_op=mybir.AluOpType.is_gt,
            fill=1.0,
            base=0,
            pattern=[[-1, 128]],
            channel_multiplier=1,
        )

        qT = inpool.tile([128, T], bf16)
        kT = inpool.tile([128, T], bf16)
        vt = inpool.tile([128, nkt_tot * 65], bf16)
        vtv = vt.rearrange("p (n k) -> p n k", k=65)
        if use_w0:
            w0 = inpool.tile([64, 128 + W0Q], bf16, padded_shape=[128, 128 + W0Q])

        # ---- input DMAs: all issued up front ----
        # kT halves are loaded separately (h0 rows 0:64 first) so head 0's QK
        # stream unblocks as early as possible; w0 covers the whole first
        # (seg0, h0) non-diagonal stream so tiles 0..3 only depend on 1-2 DMAs
        s00, s01 = segments[0]
        kg = 0
        if use_w0:
            nc.sync.dma_start(w0, w0_d)
        first = True
        for (s0, s1) in segments:
            nkt = ceil((s1 - s0) / 128)
            nc.sync.dma_start(kT[0:64, s0:s1], kT_d[:, s0:s1])
            if first and use_w0:
                nc.sync.dma_start(
                    vt[:, kg * 65 : (kg + nkt) * 65],
                    vt_d[:, kg * 65 : (kg + nkt) * 65],
                )
                nc.sync.dma_start(qT[:, s0:s1], qT_d[:, s0:s1])
            else:
                nc.sync.dma_start(qT[:, s0:s1], qT_d[:, s0:s1])
                nc.sync.dma_start(
                    vt[:, kg * 65 : (kg + nkt) * 65],
                    vt_d[:, kg * 65 : (kg + nkt) * 65],
                )
            nc.sync.dma_start(kT[64:128, s0:s1], kT_d[:, s0:s1])
            kg += nkt
            first = False

        # ---- main stream ----
        se_tiles = {}
        oh_tiles = {}
        osb_tiles = {}

        def get_oh(h, si, f):
            key = (h, si, f)
            if key not in oh_tiles:
                oh_tiles[key] = ps_o.tile(
                    [128, 512], f32, tag="oh", name=f"oh_{h}_{si}_{f}"
                )
            return oh_tiles[key]

        def get_osb(si):
            if si not in osb_tiles:
                nqc = ceil((segments[si][1] - segments[si][0]) / 128)
                osb_tiles[si] = opool.tile(
                    [128, nqc * 128], f32, tag="osb", name=f"osb_{si}"
                )
            return osb_tiles[si]

        def emit_qk(g, sp):
            # diagonal subpieces first: the DVE exp2 pass only needs those,
            # so it can start while the rest of the tile's QKs still run
            dcols = set()
            for (c0, w) in tiles[g]["ts"]:
                dcols.update(range(c0, c0 + w, 128))
            tiles[g]["qk"].sort(key=lambda p: (p[6] not in dcols, p[6]))
            for (h, s0, klo, kp, qa, qb, col, from_w0) in tiles[g]["qk"]:
                lhsT = (
                    w0[0:64, 0:kp]
                    if (from_w0 and klo == 0)
                    else kT[64 * h : 64 * h + 64, s0 + klo : s0 + klo + kp]
                )
                rhs = (
                    w0[0:64, 128 + qa : 128 + qb]
                    if from_w0
                    else qT[64 * h : 64 * h + 64, s0 + qa : s0 + qb]
                )
                nc.tensor.matmul(
                    sp[:kp, col : col + qb - qa],
                    lhsT,
                    rhs,
                    start=True,
                    stop=True,
                    tile_position=(64 * h, 0),
                )

        def emit_post(g):
            t = tiles[g]
            se = se_tiles.pop(g)
            for (h, si, qc, col, cw, ktg, kp, start, stop) in t["pv"]:
                f, fbase = qc_bank[(h, si, qc)]
                oh = get_oh(h, si, f)
                lq = qc - fbase
                nc.tensor.matmul(
                    oh[:cw, 128 * lq : 128 * lq + 65],
                    se[:kp, col : col + cw],
                    vtv[:kp, ktg, :],
                    start=start,
                    stop=stop,
                )
            for (h, si, f, qc0, nq) in t["norms"]:
                oh = oh_tiles[(h, si, f)]
                osb = get_osb(si)
                lq = 0  # bank-local offset: each norm group is a whole oh tile
                uniq = f"{h}_{si}_{qc0}"
                rcp = rpool.tile([128, 8], f32, tag="rcp", name=f"rcp_{uniq}")
                nc.vector.reciprocal(
                    rcp[:, :nq],
                    oh[:, 128 * lq + 64 : 128 * (lq + nq - 1) + 65 : 128],
                )
                ohv = oh.rearrange("p (c k) -> p c k", k=128)[:, lq : lq + nq, 0:D]
                rv = rcp[:, :nq].rearrange("p (c k) -> p c k", k=1)
                rv, ohv = bass.broadcast_tensor_aps(rv, ohv)
                nc.vector.tensor_mul(
                    osb.rearrange("p (c k) -> p c k", k=128)[
                        :, qc0 : qc0 + nq, D * h : D * h + D
                    ],
                    ohv,
                    rv,
                )
            for (si, c0, c1) in t["dmas"]:
                s0 = segments[si][0]
                osb = osb_tiles[si]
                L = segments[si][1] - s0
                r0, r1 = 128 * c0, min(128 * c1, L)
                nfc = (r1 - r0) // 128
                if nfc:
                    nc.sync.dma_start(
                        o_d[s0 + r0 : s0 + r0 + nfc * 128, :].rearrange(
                            "(c p) k -> p c k", p=128
                        ),
                        osb.rearrange("p (c k) -> p c k", k=128)[:, c0 : c0 + nfc, :],
                    )
                if (r1 - r0) % 128:
                    rr = r0 + nfc * 128
                    nc.sync.dma_start(
                        o_d[s0 + rr : s0 + r1, :],
                        osb[: r1 - rr, 128 * (c0 + nfc) : 128 * (c0 + nfc) + G * D],
                    )

        # Schraudolph exp2: exp(SCALE*s) = 2^(SCALE*s*log2e), assembled as
        # an int16 bit pattern that IS the bf16 weight: i16 = trunc(t*2^7 +
        # ((127<<7) - C)). ~3% max weight error; used on diagonal blocks
        # (query chunk 0 stays exact on ScalarE) and a slice of non-diagonal
        # cols to balance the ScalarE and Vector engines.
        LOG2E = 1.4426950408889634
        A_TS = float(SCALE * LOG2E * 128.0)
        B_TS = float((127 << 7) - 2.8)
        i16dt = mybir.dt.int16

        for g in range(len(tiles)):
            t = tiles[g]
            sz = t["sz"]
            if t["kind"] == "a":
                sp = ps_a.tile([128, TILE_A], f32, tag="spa", name=f"spa_{g}")
                se = sepool.tile([128, TILE_A], bf16, tag="se", name=f"se_{g}")
            else:
                sp = ps_d.tile([128, TILE_D], f32, tag="spd", name=f"spd_{g}")
                se = sdpool.tile([128, TILE_D], bf16, tag="sed", name=f"sed_{g}")
            emit_qk(g, sp)
            se_tiles[g] = se
            # DVE exp2 bit trick straight into bf16: i16 = t*2^7 + bias is
            # the bit pattern of ~2^t in bf16 (one tensor_scalar, no copy)
            for (c0, w) in t["ts"]:
                nc.vector.tensor_scalar(
                    se[:, c0 : c0 + w].bitcast(i16dt),
                    sp[:, c0 : c0 + w],
                    A_TS,
                    B_TS,
                    mybir.AluOpType.mult,
                    mybir.AluOpType.add,
                )
            # causal mask for diagonal chunks (cheap bf16 multiply; Pool for
            # mid-stream tiles, DVE for the latency-critical closing tiles)
            for (c0, n, fm) in t["masks"]:
                sev = se[:, c0 : c0 + 128 * n].rearrange("p (c k) -> p c k", k=128)
                triv = trimask.rearrange("p (c k) -> p c k", k=128)
                triv, sev = bass.broadcast_tensor_aps(triv, sev)
                eng = nc.vector if fm else nc.gpsimd
                eng.tensor_mul(
                    se[:, c0 : c0 + 128 * n].rearrange("p (c k) -> p c k", k=128),
                    sev,
                    triv,
                )
            if g >= 3:
                emit_post(g - 3)
            # ScalarE exp over the complement of the DVE ranges
            pos = 0
            acts = []
            for (c0, w) in t["ts"]:
                if c0 > pos:
                    acts.append((pos, c0))
                pos = c0 + w
            if pos < sz:
                acts.append((pos, sz))
            for (a, b) in acts:
                nc.scalar.activation(
                    se[:, a:b],
                    sp[:, a:b],
                    mybir.ActivationFunctionType.Exp,
                    scale=SCALE,
                )
            # mask for ScalarE-path diagonal chunks (query chunk 0)
            for (c, fm) in t["pmask"]:
                eng = nc.vector if fm else nc.gpsimd
                eng.tensor_mul(se[:, c : c + 128], se[:, c : c + 128], trimask)
        emit_post(len(tiles) - 3)
        emit_post(len(tiles) - 2)
        emit_post(len(tiles) - 1)

    nc.compile()
    return nc


def _segments_from_cu(cu_seqlens, T):
    edges = sorted(set([0, T] + [int(c) for c in cu_seqlens if 0 < int(c) < T]))
    return [(edges[i], edges[i + 1]) for i in range(len(edges) - 1)]


def kernel(q, k, v, k_cache, v_cache, slot_mapping, cu_seqlens):
    global LAST_RESULT
    T = q.shape[0]
    nslots = k_cache.shape[0]

    # Emulate scatter-then-gather through the paged cache: for duplicate slots
    # the last writer wins, so token i reads back k[lastw[slot[i]]].
    slot = np.asarray(slot_mapping, dtype=np.int64)
    lastw = np.zeros(nslots, dtype=np.int64)
    lastw[slot] = np.arange(T)
    lw = lastw[slot]
    k_eff = np.asarray(k)[lw]
    v_eff = np.asarray(v)[lw]

    segments = _segments_from_cu(np.asarray(cu_seqlens), T)
    key = (T, tuple(segments))
    if key not in _PROGRAM_CACHE:
        _PROGRAM_CACHE[key] = _build_program(T, segments)
    nc = _PROGRAM_CACHE[key]

    bf = ml_dtypes.bfloat16
    qh = np.ascontiguousarray(
        np.asarray(q).reshape(T, NKV * G, D).transpose(1, 2, 0)
    ).astype(bf)  # [16, 64, T]
    kh = np.ascontiguousarray(k_eff.reshape(T, NKV, D).transpose(1, 2, 0)).astype(bf)
    vh = v_eff.reshape(T, NKV, D).astype(bf)  # [T, 8, 64]

    # vt: per-segment 128-row tiling of v rows, with a ones column at k=64
    nkt_tot = sum(ceil((s1 - s0) / 128) for (s0, s1) in segments)
    L0 = segments[0][1] - segments[0][0]
    use_w0 = L0 >= 1024

    in_maps = []
    for h in range(NKV):
        qT = np.ascontiguousarray(qh[2 * h : 2 * h + 2].reshape(128, T))
        kT = np.ascontiguousarray(kh[h])  # [64, T]
        vt = np.zeros((128, nkt_tot, 65), dtype=bf)
        kg = 0
        for (s0, s1) in segments:
            L = s1 - s0
            for kt in range(ceil(L / 128)):
                klo = s0 + 128 * kt
                kp = min(128, s1 - klo)
                vt[:kp, kg, :D] = vh[klo : klo + kp, h, :]
                vt[:, kg, D] = 1.0
                kg += 1
        m = {
            "qT": qT,
            "kT": kT,
            "vt": np.ascontiguousarray(vt.reshape(128, nkt_tot * 65)),
        }
        if use_w0:
            s00 = segments[0][0]
            w0 = np.concatenate(
                [kT[:, s00 : s00 + 128], qT[0:64, s00 : s00 + W0Q]], axis=1
            )
            m["w0"] = np.ascontiguousarray(w0)
        in_maps.append(m)

    res = run_bass_kernel_spmd(nc, in_maps, core_ids=list(range(8)), trace=TRACE)
    LAST_RESULT = res

    out = np.empty((T, NKV * G * D), dtype=np.float32)
    ov = out.reshape(T, NKV, G * D)
    for h in range(NKV):
        ov[:, h, :] = res.results[h]["o"]
    return out


# revision 62
# speedup vs baseline: 1.5168x; 1.0459x over previous
"""Paged-KV varlen causal GQA attention for Trainium2, sharded over 8 NeuronCores.

Problem (hardcoded from spec): T=4096 tokens, 16 q heads / 8 kv heads, head_dim=64,
scale=0.125. k/v are scattered into paged caches via slot_mapping, read back, and
causal varlen attention (segments from cu_seqlens) is computed.

Sharding: tensor-parallel over kv heads -- core h gets kv head h and its 2 GQA
query heads. slot_mapping / cu_seqlens handled on host (index math only).

Device kernel (per core): TWO independent score-tile streams (so each exp
engine paces its own PSUM double-buffer):
  A stream [128,1024] f32 x2 bufs:  se = exp(0.125*sp) on ScalarE
  D stream [128, 512] f32 x2 bufs:  se = 2^(0.125*sp*log2e) on VectorE via the
      Schraudolph int16 bit trick written straight into bf16 (~3% weight err;
      carries the diagonal blocks except query-chunk 0, plus enough late
      non-diagonal stripes to balance ScalarE vs VectorE)
  sp[keys, queries] = kT.T @ qT          (PE, per piece, bank-split)
  diagonal blocks: se *= trimask         (GpSimd mid-stream, VectorE at the tail)
  o[q, 0:65] += se[:,qc].T @ [v | 1]     (PE with se as STATIONARY: output lands
                                          query-major, col 64 = softmax denom --
                                          no transposes / PSUM->SBUF copies;
                                          accumulation start/stop flags are per
                                          PSUM bank, set by the first/last PV)
  osb[q, d] = o[q, d] * 1/o[q, 64]       (VectorE reciprocal + broadcast mul)
"""

import os
from contextlib import ExitStack
from math import ceil

import numpy as np
import ml_dtypes

import concourse.bass as bass
import concourse.mybir as mybir
import concourse.tile as tile
from concourse import bacc
from concourse.bass_utils import run_bass_kernel_spmd

NKV = 8
G = 2
D = 64
SCALE = 0.125

TILE_A = 1024  # ScalarE score tile width (2 PSUM banks of f32, 2 buffers)
TILE_D = 512   # DVE exp2 score tile width (1 PSUM bank, 2 buffers)
BANK = 512     # f32 cols per PSUM bank
W0Q = 1024     # warmup tensor covers kT[0:128] + qT[0:1024] of segment 0

# test.py pokes these for profiling
TRACE = bool(int(os.environ.get("KERNEL_TRACE", "0")))
LAST_RESULT = None

_PROGRAM_CACHE = {}


def _plan(segments):
    """Pack the per-(segment, head) score work into TWO independent tile
    streams, interleaved in one creation-order list:
      kind 'a': ScalarE-exp tiles, [128, <=1024] f32 (2 PSUM banks x 2 bufs)
      kind 'd': DVE-exp2 tiles,    [128, <=512]  f32 (1 PSUM bank  x 2 bufs)
    Each engine paces its own stream, so an sp slot is freed by exactly one
    reader and the pipelines decouple.

    Tile dicts: kind, sz, qk, ts, masks, pmask, pv, norms, dmas.
    """
    tiles = []
    cur = {"a": None, "d": None}
    acap = [384, 640]  # warmup caps for the first A tiles, then TILE_A
    chains = {}  # (h, si, qc) -> [ [tile_idx, col, cw, ktg, kp], ... ]

    def new_tile(kind):
        t = {
            "kind": kind, "idx": len(tiles), "sz": 0, "qk": [], "ts": [],
            "masks": [], "pmask": [], "pv": [], "norms": [], "dmas": [],
        }
        tiles.append(t)
        cur[kind] = t
        return t

    def close(kind):
        cur[kind] = None

    def cap(kind):
        if kind == "d":
            return TILE_D
        na = sum(1 for t in tiles if t["kind"] == "a")
        if cur["a"] is not None:
            na -= 1
        return acap[na] if na < len(acap) else TILE_A

    # global key-tile counter (vt layout is per-segment tiled on host)
    seg_ktg0 = []
    nkt_tot = 0
    for (s0, s1) in segments:
        seg_ktg0.append(nkt_tot)
        nkt_tot += ceil((s1 - s0) / 128)

    def place_span(kind, h, si, kt, qa, qb, w0_ok):
        """Place one query span of key tile kt into the `kind` stream,
        splitting at tile caps and PSUM bank edges; register PV chunks.
        Returns [(tile, col, ncols)] placements."""
        s0, s1 = segments[si]
        L = s1 - s0
        klo = 128 * kt
        kp = min(128, L - klo)
        ktg = seg_ktg0[si] + kt
        placements = []
        q = qa
        while q < qb:
            if cur[kind] is None or cur[kind]["sz"] >= cap(kind):
                new_tile(kind)
            t = cur[kind]
            room = cap(kind) - t["sz"]
            take = min(qb - q, room)
            col = t["sz"]
            placements.append((t, col, take))
            qq = q
            while qq < q + take:
                c = col + (qq - q)
                lim = qq + min(q + take - qq, BANK - (c % BANK))
                use_w0 = w0_ok and qq < W0Q
                if use_w0 and lim > W0Q:
                    lim = W0Q
                t["qk"].append((h, s0, klo, kp, qq, lim, c, use_w0))
                qq = lim
            for qs in range(q, q + take, 128):
                qc = qs // 128
                cw = min(128, qb - qs, q + take - qs)
                chains.setdefault((h, si, qc), []).append(
                    [t["idx"], col + (qs - q), cw, ktg, kp]
                )
            t["sz"] += take
            if t["sz"] % 128:
                t["sz"] += 128 - t["sz"] % 128
            q += take
        return placements

    def place_diag(h, si, kts, act, w0_ok=False, fast_mask=False):
        """Diagonal chunks: kt0 (act=True) goes on the A stream with a
        mask-multiply (pmask); others go on the D stream (DVE exp2) with
        the causal mask applied by Pool (or DVE when fast_mask)."""
        s0, s1 = segments[si]
        L = s1 - s0
        for kt in kts:
            klo = 128 * kt
            kp = min(128, L - klo)
            pl = place_span("a" if act else "d", h, si, kt, klo, klo + kp, w0_ok)
            for (t, col, n) in pl:
                if act:
                    t["pmask"].append((col, fast_mask))
                else:
                    t["ts"].append((col, n))
                    t["masks"].append((col, n // 128, fast_mask))

    def take_nd(kind, h, si, ndq, ncols, w0_ok=False):
        """Consume ncols of non-diagonal pieces into the given stream; D-
        stream cols get ts entries (DVE exp2, no mask needed)."""
        left = ncols
        while left > 0:
            kt, qa, qb = ndq[0]
            take = min(qb - qa, left)
            pl = place_span(kind, h, si, kt, qa, qa + take, w0_ok)
            if kind == "d":
                for (t, col, n) in pl:
                    t["ts"].append((col, n))
            if qa + take == qb:
                ndq.pop(0)
            else:
                ndq[0][1] = qa + take
            left -= take

    nseg = len(segments)
    for si, (s0, s1) in enumerate(segments):
        L = s1 - s0
        nkt = ceil(L / 128)
        nqc = ceil(L / 128)
        for h in range(G):
            first_sh = si == 0 and h == 0 and L >= 1024
            last_sh = (
                si == nseg - 1 and h == G - 1 and nqc >= 8 and L % 128 == 0
            )
            nd = [kt for kt in range(nkt - 1) if 128 * (kt + 1) < L]
            fast = L == 1024 and nkt == 8
            if fast and not last_sh:
                w0_ok = first_sh
                ndq = [[kt, 128 * (kt + 1), L] for kt in nd]
                # route the last 640 nd cols (late stripes of kt4..6, well
                # diluted query chunks) through the DVE stream; with the kt0
                # diagonal on the A stream both streams then hold an exact
                # number of tiles per (seg, head), which keeps oh-bank norms
                # strictly before the next tenant's first PV in program order
                extras = [[4, 768, 1024], [5, 768, 1024], [6, 896, 1024]]
                ndq[4][2] = 768
                ndq[5][2] = 768
                ndq = [p for p in ndq if p[1] < p[2] and p[0] != 6]
                take_nd("a", h, si, ndq, 896, w0_ok)
                place_diag(h, si, [0], True, w0_ok)
                place_diag(h, si, [1, 2, 3, 4], False, w0_ok)
                take_nd("a", h, si, ndq, 1024, w0_ok)
                place_diag(h, si, [5, 6, 7], False, w0_ok)
                take_nd("d", h, si, extras, 640, w0_ok)
                take_nd("a", h, si, ndq, 1024, w0_ok)
                close("a")
                close("d")
            elif fast and last_sh:
                # tail: qchunk 7's pieces and the final diagonal chunk come
                # last, in their own small tiles, for a short closing chain
                ndqa = [[kt, 128 * (kt + 1), L - 128] for kt in nd]
                ndqb = [[kt, L - 128, L] for kt in nd]
                extras = [[4, 640, 896], [5, 768, 896]]
                ndqa[4][2] = 640
                ndqa[5] = [5, 768, 768]
                ndqa = [p for p in ndqa if p[1] < p[2]]
                take_nd("a", h, si, ndqa, 1024)
                place_diag(h, si, [1, 2, 3], False, fast_mask=True)
                take_nd("a", h, si, ndqa, 1024)
                place_diag(h, si, [4, 5, 6], False, fast_mask=True)
                take_nd("d", h, si, extras, 384)
                take_nd("a", h, si, ndqa, 256)
                place_diag(h, si, [0], True, fast_mask=True)
                close("a")
                take_nd("a", h, si, ndqb, 896)
                close("a")
                close("d")
                place_diag(h, si, [7], False, fast_mask=True)
                close("d")
            else:
                # generic fallback: everything on the A stream, diag last,
                # masks on Pool
                for kt in nd:
                    place_span("a", h, si, kt, 128 * (kt + 1), L, first_sh)
                place_diag(h, si, [0], True, first_sh)
                place_diag(h, si, list(range(1, nkt)), False, first_sh)

    # coalesce adjacent ts / mask ranges per tile
    for t in tiles:
        for key in ("ts",):
            t[key].sort()
            merged = []
            for (c0, w) in t[key]:
                if merged and merged[-1][0] + merged[-1][1] == c0:
                    merged[-1][1] += w
                else:
                    merged.append([c0, w])
            t[key] = [tuple(m) for m in merged]
        t["masks"].sort()
        merged = []
        for (c0, n, fm) in t["masks"]:
            if merged and merged[-1][0] + 128 * merged[-1][1] == c0 and merged[-1][2] == fm:
                merged[-1][1] += n
            else:
                merged.append([c0, n, fm])
        t["masks"] = [tuple(m) for m in merged]

    # qchunk -> oh-bank mapping: 4 chunks per PSUM bank, except the last
    # (segment, head) where the final chunk gets its own bank so its
    # accumulation group closes independently
    def oh_banks(h, si):
        L = segments[si][1] - segments[si][0]
        nqc = ceil(L / 128)
        if si == nseg - 1 and h == G - 1 and nqc == 8:
            return [[0, 1, 2], [3, 4, 5, 6], [7]]
        return [
            list(range(4 * f, min(4 * f + 4, nqc))) for f in range(ceil(nqc / 4))
        ]

    qc_bank = {}  # (h, si, qc) -> (f, fbase)
    for si in range(nseg):
        for h in range(G):
            for f, qcs in enumerate(oh_banks(h, si)):
                for qc in qcs:
                    qc_bank[(h, si, qc)] = (f, qcs[0])

    # start/stop flags are PER PSUM BANK (zero region), not per qchunk chain:
    # start_tensor_calc pends-zero the whole 2KB bank, so only the
    # chronologically first matmul into an oh tile may set it, and only the
    # last sets stop. Per-byte lazy zeroing makes each chain's first write a
    # plain store and later writes accumulates, regardless of interleaving.
    oh_groups = {}
    for (h, si, qc), ch in chains.items():
        for e in ch:
            oh_groups.setdefault((h, si, qc_bank[(h, si, qc)][0]), []).append(e)
    for grp in oh_groups.values():
        grp.sort(key=lambda e: (e[0], e[1]))
        for e in grp:
            e.append(e is grp[0])
            e.append(e is grp[-1])
    for (h, si, qc), ch in chains.items():
        for e in ch:
            tiles[e[0]]["pv"].append((h, si, qc, e[1], e[2], e[3], e[4], e[5], e[6]))

    # order pv entries in each tile by emission col to keep per-chain order
    for t in tiles:
        t["pv"].sort(key=lambda p: (p[3], p[0], p[1], p[2]))

    # norm groups (one per oh bank): due after max last tile over the chains
    for si in range(nseg):
        norm_due = {}
        for h in range(G):
            for f, gq in enumerate(oh_banks(h, si)):
                last = max(chains[(h, si, qc)][-1][0] for qc in gq)
                tiles[last]["norms"].append((h, si, f, gq[0], len(gq)))
                for qc in gq:
                    norm_due[(h, qc)] = last
        row_groups = oh_banks(G - 1, si)
        if len(row_groups) > 2:
            for gq in row_groups:
                due = max(norm_due[(h, qc)] for h in range(G) for qc in gq)
                tiles[due]["dmas"].append((si, gq[0], gq[-1] + 1))
        else:
            due = max(norm_due.values())
            nqc = ceil((segments[si][1] - segments[si][0]) / 128)
            tiles[due]["dmas"].append((si, 0, nqc))

    return tiles, nkt_tot, qc_bank


def _build_program(T, segments):
    f32 = mybir.dt.float32
    bf16 = mybir.dt.bfloat16

    tiles, nkt_tot, qc_bank = _plan(segments)
    L0 = segments[0][1] - segments[0][0]
    use_w0 = L0 >= 1024

    nc = bacc.Bacc(
        "TRN2",
        target_bir_lowering=False,
        debug=False,
        enable_asserts=False,
        num_devices=8,
    )
    qT_d = nc.dram_tensor("qT", [128, T], bf16, kind="ExternalInput").ap()
    kT_d = nc.dram_tensor("kT", [64, T], bf16, kind="ExternalInput").ap()
    vt_d = nc.dram_tensor("vt", [128, nkt_tot * 65], bf16, kind="ExternalInput").ap()
    if use_w0:
        w0_d = nc.dram_tensor("w0", [64, 128 + W0Q], bf16, kind="ExternalInput").ap()
    o_d = nc.dram_tensor("o", [T, G * D], f32, kind="ExternalOutput").ap()

    with tile.TileContext(nc) as tc, ExitStack() as ctx:
        const = ctx.enter_context(tc.tile_pool(name="const", bufs=1))
        inpool = ctx.enter_context(tc.tile_pool(name="inp", bufs=1))
        sepool = ctx.enter_context(tc.tile_pool(name="se", bufs=10))
        sdpool = ctx.enter_context(tc.tile_pool(name="sed", bufs=10))
        opool = ctx.enter_context(tc.tile_pool(name="osb", bufs=3))
        rpool = ctx.enter_context(tc.tile_pool(name="rcp", bufs=6))
        ps_a = ctx.enter_context(tc.tile_pool(name="ps_a", bufs=2, space="PSUM"))
        ps_d = ctx.enter_context(tc.tile_pool(name="ps_d", bufs=2, space="PSUM"))
        ps_o = ctx.enter_context(tc.tile_pool(name="ps_o", bufs=2, space="PSUM"))

        # trimask[p, c] = 1 if c >= p else 0 (valid = query col >= key partition)
        trimask = const.tile([128, 128], bf16)
        nc.gpsimd.memset(trimask, 0.0)
        nc.gpsimd.affine_select(
            out=trimask,
            in_=trimask,
            compare_op=mybir.AluOpType.is_gt,
            fill=1.0,
            base=0,
            pattern=[[-1, 128]],
            channel_multiplier=1,
        )

        qT = inpool.tile([128, T], bf16)
        kT = inpool.tile([128, T], bf16)
        vt = inpool.tile([128, nkt_tot * 65], bf16)
        vtv = vt.rearrange("p (n k) -> p n k", k=65)
        if use_w0:
            w0 = inpool.tile([64, 128 + W0Q], bf16, padded_shape=[128, 128 + W0Q])

        # ---- input DMAs: all issued up front ----
        # kT halves are loaded separately (h0 rows 0:64 first) so head 0's QK
        # stream unblocks as early as possible; w0 covers the whole first
        # (seg0, h0) non-diagonal stream so tiles 0..3 only depend on 1-2 DMAs
        s00, s01 = segments[0]
        kg = 0
        if use_w0:
            nc.sync.dma_start(w0, w0_d)
        first = True
        for (s0, s1) in segments:
            nkt = ceil((s1 - s0) / 128)
            nc.sync.dma_start(kT[0:64, s0:s1], kT_d[:, s0:s1])
            if first and use_w0:
                nc.sync.dma_start(qT[:, s0:s1], qT_d[:, s0:s1])
                nc.sync.dma_start(
                    vt[:, kg * 65 : (kg + nkt) * 65],
                    vt_d[:, kg * 65 : (kg + nkt) * 65],
                )
            else:
                nc.sync.dma_start(qT[:, s0:s1], qT_d[:, s0:s1])
                nc.sync.dma_start(
                    vt[:, kg * 65 : (kg + nkt) * 65],
                    vt_d[:, kg * 65 : (kg + nkt) * 65],
                )
            nc.sync.dma_start(kT[64:128, s0:s1], kT_d[:, s0:s1])
            kg += nkt
            first = False

        # ---- main stream ----
        se_tiles = {}
        oh_tiles = {}
        osb_tiles = {}

        def get_oh(h, si, f):
            key = (h, si, f)
            if key not in oh_tiles:
                oh_tiles[key] = ps_o.tile(
                    [128, 512], f32, tag="oh", name=f"oh_{h}_{si}_{f}"
                )
            return oh_tiles[key]

        def get_osb(si):
            if si not in osb_tiles:
                nqc = ceil((segments[si][1] - segments[si][0]) / 128)
                osb_tiles[si] = opool.tile(
                    [128, nqc * 128], f32, tag="osb", name=f"osb_{si}"
                )
            return osb_tiles[si]

        def emit_qk(g, sp):
            # diagonal subpieces first: the DVE exp2 pass only needs those,
            # so it can start while the rest of the tile's QKs still run
            dcols = set()
            for (c0, w) in tiles[g]["ts"]:
                dcols.update(range(c0, c0 + w, 128))
            tiles[g]["qk"].sort(key=lambda p: (p[6] not in dcols, p[6]))
            for (h, s0, klo, kp, qa, qb, col, from_w0) in tiles[g]["qk"]:
                lhsT = (
                    w0[0:64, 0:kp]
                    if (from_w0 and klo == 0)
                    else kT[64 * h : 64 * h + 64, s0 + klo : s0 + klo + kp]
                )
                rhs = (
                    w0[0:64, 128 + qa : 128 + qb]
                    if from_w0
                    else qT[64 * h : 64 * h + 64, s0 + qa : s0 + qb]
                )
                nc.tensor.matmul(
                    sp[:kp, col : col + qb - qa],
                    lhsT,
                    rhs,
                    start=True,
                    stop=True,
                    tile_position=(64 * h, 0),
                )

        def emit_post(g):
            t = tiles[g]
            se = se_tiles.pop(g)
            for (h, si, qc, col, cw, ktg, kp, start, stop) in t["pv"]:
                f, fbase = qc_bank[(h, si, qc)]
                oh = get_oh(h, si, f)
                lq = qc - fbase
                nc.tensor.matmul(
                    oh[:cw, 128 * lq : 128 * lq + 65],
                    se[:kp, col : col + cw],
                    vtv[:kp, ktg, :],
                    start=start,
                    stop=stop,
                )
            for (h, si, f, qc0, nq) in t["norms"]:
                oh = oh_tiles[(h, si, f)]
                osb = get_osb(si)
                lq = 0  # bank-local offset: each norm group is a whole oh tile
                uniq = f"{h}_{si}_{qc0}"
                rcp = rpool.tile([128, 8], f32, tag="rcp", name=f"rcp_{uniq}")
                nc.vector.reciprocal(
                    rcp[:, :nq],
                    oh[:, 128 * lq + 64 : 128 * (lq + nq - 1) + 65 : 128],
                )
                ohv = oh.rearrange("p (c k) -> p c k", k=128)[:, lq : lq + nq, 0:D]
                rv = rcp[:, :nq].rearrange("p (c k) -> p c k", k=1)
                rv, ohv = bass.broadcast_tensor_aps(rv, ohv)
                nc.vector.tensor_mul(
                    osb.rearrange("p (c k) -> p c k", k=128)[
                        :, qc0 : qc0 + nq, D * h : D * h + D
                    ],
                    ohv,
                    rv,
                )
            for (si, c0, c1) in t["dmas"]:
                s0 = segments[si][0]
                osb = osb_tiles[si]
                L = segments[si][1] - s0
                r0, r1 = 128 * c0, min(128 * c1, L)
                nfc = (r1 - r0) // 128
                if nfc:
                    nc.sync.dma_start(
                        o_d[s0 + r0 : s0 + r0 + nfc * 128, :].rearrange(
                            "(c p) k -> p c k", p=128
                        ),
                        osb.rearrange("p (c k) -> p c k", k=128)[:, c0 : c0 + nfc, :],
                    )
                if (r1 - r0) % 128:
                    rr = r0 + nfc * 128
                    nc.sync.dma_start(
                        o_d[s0 + rr : s0 + r1, :],
                        osb[: r1 - rr, 128 * (c0 + nfc) : 128 * (c0 + nfc) + G * D],
                    )

        # Schraudolph exp2: exp(SCALE*s) = 2^(SCALE*s*log2e), assembled as
        # an int16 bit pattern that IS the bf16 weight: i16 = trunc(t*2^7 +
        # ((127<<7) - C)). ~3% max weight error; used on diagonal blocks
        # (query chunk 0 stays exact on ScalarE) and a slice of non-diagonal
        # cols to balance the ScalarE and Vector engines.
        LOG2E = 1.4426950408889634
        A_TS = float(SCALE * LOG2E * 128.0)
        B_TS = float((127 << 7) - 2.8)
        i16dt = mybir.dt.int16

        for g in range(len(tiles)):
            t = tiles[g]
            sz = t["sz"]
            if t["kind"] == "a":
                sp = ps_a.tile([128, TILE_A], f32, tag="spa", name=f"spa_{g}")
                se = sepool.tile([128, TILE_A], bf16, tag="se", name=f"se_{g}")
            else:
                sp = ps_d.tile([128, TILE_D], f32, tag="spd", name=f"spd_{g}")
                se = sdpool.tile([128, TILE_D], bf16, tag="sed", name=f"sed_{g}")
            emit_qk(g, sp)
            se_tiles[g] = se
            # DVE exp2 bit trick straight into bf16: i16 = t*2^7 + bias is
            # the bit pattern of ~2^t in bf16 (one tensor_scalar, no copy)
            for (c0, w) in t["ts"]:
                nc.vector.tensor_scalar(
                    se[:, c0 : c0 + w].bitcast(i16dt),
                    sp[:, c0 : c0 + w],
                    A_TS,
                    B_TS,
                    mybir.AluOpType.mult,
                    mybir.AluOpType.add,
                )
            # causal mask for diagonal chunks (cheap bf16 multiply; Pool for
            # mid-stream tiles, DVE for the latency-critical closing tiles)
            for (c0, n, fm) in t["masks"]:
                sev = se[:, c0 : c0 + 128 * n].rearrange("p (c k) -> p c k", k=128)
                triv = trimask.rearrange("p (c k) -> p c k", k=128)
                triv, sev = bass.broadcast_tensor_aps(triv, sev)
                eng = nc.vector if fm else nc.gpsimd
                eng.tensor_mul(
                    se[:, c0 : c0 + 128 * n].rearrange("p (c k) -> p c k", k=128),
                    sev,
                    triv,
                )
            if g >= 6:
                emit_post(g - 6)
            # ScalarE exp over the complement of the DVE ranges
            pos = 0
            acts = []
            for (c0, w) in t["ts"]:
                if c0 > pos:
                    acts.append((pos, c0))
                pos = c0 + w
            if pos < sz:
                acts.append((pos, sz))
            for (a, b) in acts:
                nc.scalar.activation(
                    se[:, a:b],
                    sp[:, a:b],
                    mybir.ActivationFunctionType.Exp,
                    scale=SCALE,
                )
            # mask for ScalarE-path diagonal chunks (query chunk 0)
            for (c, fm) in t["pmask"]:
                eng = nc.vector if fm else nc.gpsimd
                eng.tensor_mul(se[:, c : c + 128], se[:, c : c + 128], trimask)
        for gg in range(max(0, len(tiles) - 6), len(tiles)):
            emit_post(gg)

    nc.compile()
    return nc


def _segments_from_cu(cu_seqlens, T):
    edges = sorted(set([0, T] + [int(c) for c in cu_seqlens if 0 < int(c) < T]))
    return [(edges[i], edges[i + 1]) for i in range(len(edges) - 1)]


def kernel(q, k, v, k_cache, v_cache, slot_mapping, cu_seqlens):
    global LAST_RESULT
    T = q.shape[0]
    nslots = k_cache.shape[0]

    # Emulate scatter-then-gather through the paged cache: for duplicate slots
    # the last writer wins, so token i reads back k[lastw[slot[i]]].
    slot = np.asarray(slot_mapping, dtype=np.int64)
    lastw = np.zeros(nslots, dtype=np.int64)
    lastw[slot] = np.arange(T)
    lw = lastw[slot]
    k_eff = np.asarray(k)[lw]
    v_eff = np.asarray(v)[lw]

    segments = _segments_from_cu(np.asarray(cu_seqlens), T)
    key = (T, tuple(segments))
    if key not in _PROGRAM_CACHE:
        _PROGRAM_CACHE[key] = _build_program(T, segments)
    nc = _PROGRAM_CACHE[key]

    bf = ml_dtypes.bfloat16
    qh = np.ascontiguousarray(
        np.asarray(q).reshape(T, NKV * G, D).transpose(1, 2, 0)
    ).astype(bf)  # [16, 64, T]
    kh = np.ascontiguousarray(k_eff.reshape(T, NKV, D).transpose(1, 2, 0)).astype(bf)
    vh = v_eff.reshape(T, NKV, D).astype(bf)  # [T, 8, 64]

    # vt: per-segment 128-row tiling of v rows, with a ones column at k=64
    nkt_tot = sum(ceil((s1 - s0) / 128) for (s0, s1) in segments)
    L0 = segments[0][1] - segments[0][0]
    use_w0 = L0 >= 1024

    in_maps = []
    for h in range(NKV):
        qT = np.ascontiguousarray(qh[2 * h : 2 * h + 2].reshape(128, T))
        kT = np.ascontiguousarray(kh[h])  # [64, T]
        vt = np.zeros((128, nkt_tot, 65), dtype=bf)
        kg = 0
        for (s0, s1) in segments:
            L = s1 - s0
            for kt in range(ceil(L / 128)):
                klo = s0 + 128 * kt
                kp = min(128, s1 - klo)
                vt[:kp, kg, :D] = vh[klo : klo + kp, h, :]
                vt[:, kg, D] = 1.0
                kg += 1
        m = {
            "qT": qT,
            "kT": kT,
            "vt": np.ascontiguousarray(vt.reshape(128, nkt_tot * 65)),
        }
        if use_w0:
            s00 = segments[0][0]
            w0 = np.concatenate(
                [kT[:, s00 : s00 + 128], qT[0:64, s00 : s00 + W0Q]], axis=1
            )
            m["w0"] = np.ascontiguousarray(w0)
        in_maps.append(m)

    res = run_bass_kernel_spmd(nc, in_maps, core_ids=list(range(8)), trace=TRACE)
    LAST_RESULT = res

    out = np.empty((T, NKV * G * D), dtype=np.float32)
    ov = out.reshape(T, NKV, G * D)
    for h in range(NKV):
        ov[:, h, :] = res.results[h]["o"]
    return out
